# revision 1
# baseline (speedup 1.0000x reference)
"""Trainium2 Bass kernel for nn_BESNumEigen3qubitModel.

Math reduction (exact):
  vec = rho_vec / ||rho_vec||;  rho = sum_i vec_i G_i + I/8  (Hermitian 8x8, trace 1)
  dm0 = beta0*(rho - I/8) + I/8, dm1 = beta1*(rho - I/8) + I/8 are AFFINE in rho,
  and partial transposes are linear, so every eigvalsh in the reference reduces
  to eigenvalues of just 3 Hermitian matrices per batch element:
     rho, pt_a(rho), pt_c(rho).
  With w = eig(rho) ascending, S_k0 = sum of k0 smallest, T_k1 = sum of k1 largest,
  mu/nu = eig extrema of pt_a/pt_c:
     beta0 = 1/(1-8 w_min), beta1 = 1/(1-8 w_max)   (beta0>0, beta1<0)
     loss0 = beta0*(S_k0 - k0/8) + k0/8 ; loss1 = beta1*(T_k1 - k1/8) + k1/8
     loss  = (loss0+loss1)^2 + sum over 4 PPT terms (beta*(ext-1/8)+1/8)^2
  where ext = mu_min (beta0), mu_max (beta1), nu_min (beta0), nu_max (beta1).

Device kernel: batched branchless complex Jacobi (4 full sweeps, XOR-pair order)
on 3*4096 = 12288 8x8 Hermitian matrices per core (batch on partitions, matrices
along free dim), then an 8-element sorting network on rho's diagonal, min/max
reduction for the PT diagonals, and the scalar loss assembly.
"""

import numpy as np

D = 8
BATCH = 32768
NCORES = 8
PER_CORE = BATCH // NCORES       # 4096
NTILES = PER_CORE // 128         # 32 batch tiles per core
NM = 3 * NTILES                  # 96 matrices per partition (type-major)

_f32 = np.float32

# ---------------------------------------------------------------- host prep --

def _gellmann_basis(d):
    mats = []
    for j in range(d):
        for k in range(j + 1, d):
            m = np.zeros((d, d), np.complex128); m[j, k] = 1; m[k, j] = 1
            mats.append(m)
    for j in range(d):
        for k in range(j + 1, d):
            m = np.zeros((d, d), np.complex128); m[j, k] = -1j; m[k, j] = 1j
            mats.append(m)
    for l in range(1, d):
        m = np.zeros((d, d), np.complex128)
        m[np.arange(l), np.arange(l)] = 1
        m[l, l] = -l
        mats.append(np.sqrt(2.0 / (l * (l + 1))) * m)
    return np.stack(mats)


def _build_maps():
    """[64, 384] f32 map: (vec, 1) -> 128 floats each of rho, pt_a(rho), pt_c(rho).

    Float layout per matrix: f in [0,64) = Re[i,j] at f=i*8+j; [64,128) = Im[i,j].
    """
    G = _gellmann_basis(D)
    B = np.zeros((64, 128), np.float64)
    for k in range(63):
        B[k, :64] = G[k].real.reshape(-1)
        B[k, 64:] = G[k].imag.reshape(-1)
    B[63, :64] = (np.eye(D) / D).reshape(-1)

    def entry_perm(kind):
        p = np.zeros(64, np.int64)
        for i in range(8):
            for j in range(8):
                if kind == 'a':
                    i2, j2 = (j & 4) | (i & 3), (i & 4) | (j & 3)
                else:
                    i2, j2 = (i & 6) | (j & 1), (j & 6) | (i & 1)
                p[i * 8 + j] = i2 * 8 + j2
        return p

    def float_perm(kind):
        e = entry_perm(kind)
        return np.concatenate([e, 64 + e])

    M3 = np.concatenate([B, B[:, float_perm('a')], B[:, float_perm('c')]], axis=1)
    return M3.astype(_f32)


_M3 = None


def _host_prep(rho_vec):
    global _M3
    if _M3 is None:
        _M3 = _build_maps()
    vec = rho_vec.astype(np.float64)
    vec = vec / np.linalg.norm(vec, axis=-1, keepdims=True)
    vec_aug = np.concatenate(
        [vec.astype(_f32), np.ones((vec.shape[0], 1), _f32)], axis=1)
    flat = vec_aug @ _M3                                   # [B, 384]
    arr = flat.reshape(NCORES, NTILES, 128, 3, 128)        # [core, t, p, type, f]
    return [np.ascontiguousarray(
        arr[c].transpose(1, 2, 0, 3).reshape(128, NM * 128)) for c in range(NCORES)]


# ------------------------------------------------------------ device kernel --

def _xor_pairs(r):
    return [(i, i ^ r) for i in range(8) if i < (i ^ r)]


# Batcher odd-even mergesort network for 8 elements (19 comparators)
_CE8 = [(0, 1), (2, 3), (4, 5), (6, 7), (0, 2), (1, 3), (4, 6), (5, 7),
        (1, 2), (5, 6), (0, 4), (1, 5), (2, 6), (3, 7), (2, 4), (3, 5),
        (1, 2), (3, 4), (5, 6)]

N_SWEEPS = 4


def _build_program(k0, k1):
    import concourse.bass as bass
    import concourse.bacc as bacc
    import concourse.mybir as mybir
    from concourse.tile import TileContext
    from contextlib import ExitStack

    f32 = mybir.dt.float32
    ALU = mybir.AluOpType
    ACT = mybir.ActivationFunctionType

    nc = bacc.Bacc("TRN2")
    mats_d = nc.dram_tensor("mats", [128, NM * 128], f32, kind="ExternalInput")
    out_d = nc.dram_tensor("out", [128, NTILES], f32, kind="ExternalOutput")

    with ExitStack() as ctx:
        tc = ctx.enter_context(TileContext(nc))
        main = ctx.enter_context(tc.tile_pool(name="main", bufs=1))
        pp = ctx.enter_context(tc.tile_pool(name="pp", bufs=3))
        cp = ctx.enter_context(tc.tile_pool(name="cp", bufs=3))

        A = main.tile([128, NM, 128], f32, name="A")
        for ch in range(8):
            nc.sync.dma_start(
                out=A[:, ch * 12:(ch + 1) * 12, :],
                in_=mats_d[:, ch * 12 * 128:(ch + 1) * 12 * 128])

        A4 = A[:].rearrange("p m (i j) -> p m i j", i=16, j=8)
        eps30 = main.tile([128, 1], f32, name="eps30")
        nc.vector.memset(eps30[:], 1e-30)
        eps35 = main.tile([128, 1], f32, name="eps35")
        nc.vector.memset(eps35[:], 1e-35)
        SH = [128, NM, 8]

        def P(tag):
            return pp.tile([128, NM], f32, tag=tag, name=tag)[:]

        def C(tag):
            return cp.tile(SH, f32, tag=tag, name=tag)[:]

        def emit_rotation(p, q, M):
            app = A4[:, 0:M, p, p]
            aqq = A4[:, 0:M, q, q]
            X = A4[:, 0:M, p, q]
            Y = A4[:, 0:M, 8 + p, q]
            SH16 = [128, M, 16]

            def PM(tag):
                return pp.tile([128, NM], f32, tag=tag, name=tag)[:][:, 0:M]

            def C16(tag):
                return cp.tile([128, NM, 16], f32, tag=tag, name=tag)

            Aap = A[:]
            pdim = list(Aap.ap[0])

            def swap_col(col):
                # [im-half; re-half] view of column `col`: [128, M, 2, 8]
                return bass.AP(tensor=Aap.tensor, offset=Aap.offset + 64 + col,
                               ap=[pdim, [128, M], [-64, 2], [8, 8]])

            sqx, sqy, n2p, g = PM("sqx"), PM("sqy"), PM("n2p"), PM("g")
            gsq, s2, h, ag = PM("gsq"), PM("s2"), PM("h"), PM("ag")
            den, T, sg, T2 = PM("den"), PM("T"), PM("sg"), PM("T2")
            t2, cden, c, u = PM("t2"), PM("cden"), PM("c"), PM("u")
            urb2, sr, si, v1 = PM("urb2"), PM("sr"), PM("si"), PM("v1")
            tb, dpp, dqq, nsr = PM("tb"), PM("dpp"), PM("dqq"), PM("nsr")
            csi_t = pp.tile([128, NM, 2], f32, tag="csi", name="csi")
            csi = csi_t[:][:, 0:M, :]

            nc.scalar.activation(sqx, X, ACT.Square, scale=2.0)
            nc.scalar.activation(sqy, Y, ACT.Square, scale=2.0)
            nc.vector.tensor_tensor(n2p, sqx, sqy, ALU.add)        # b'^2 = 4|apq|^2
            nc.vector.tensor_tensor(g, app, aqq, ALU.subtract)     # g' = app - aqq
            nc.scalar.square(gsq, g)
            nc.vector.tensor_tensor(s2, gsq, n2p, ALU.add)
            nc.scalar.activation(h, s2, ACT.Sqrt, bias=eps30[:])   # sqrt(g^2+b'^2)
            nc.scalar.activation(ag, g, ACT.Abs)
            nc.vector.tensor_tensor(den, ag, h, ALU.add)
            nc.vector.reciprocal(T, den)                           # 1/(|g|+h)
            nc.scalar.sign(sg, g, bias=eps35[:])                   # sign(g), 0 -> +1
            nc.gpsimd.tensor_tensor(T2, T, T, ALU.mult)
            nc.gpsimd.tensor_tensor(t2, n2p, T2, ALU.mult)         # t^2
            nc.scalar.activation(cden, t2, ACT.Sqrt, bias=1.0)     # sqrt(1+t^2)
            nc.vector.reciprocal(c, cden)                          # cos
            nc.gpsimd.tensor_tensor(u, T, sg, ALU.mult)
            nc.vector.scalar_tensor_tensor(urb2, u, 2.0, c, ALU.mult, ALU.mult)
            nc.gpsimd.tensor_tensor(sr, urb2, X, ALU.mult)
            nc.gpsimd.tensor_tensor(si, urb2, Y, ALU.mult)
            nc.vector.tensor_tensor(v1, T, n2p, ALU.mult)
            nc.vector.scalar_tensor_tensor(tb, v1, 0.5, sg, ALU.mult, ALU.mult)
            nc.gpsimd.tensor_tensor(dpp, app, tb, ALU.add)
            nc.gpsimd.tensor_tensor(dqq, aqq, tb, ALU.subtract)
            nc.scalar.activation(nsr, sr, ACT.Copy, scale=-1.0)
            nc.gpsimd.tensor_copy(csi[:, :, 0], si)
            nc.scalar.activation(csi[:, :, 1], si, ACT.Copy, scale=-1.0)

            Ap16 = A4[:, 0:M, 0:16, p]
            Aq16 = A4[:, 0:M, 0:16, q]
            Aqsw = swap_col(q)
            cp16_t, P1_t, P2_t = C16("cp16"), C16("P1"), C16("P2")
            Q1_t, Q2_t = C16("Q1"), C16("Q2")
            cp16 = cp16_t[:][:, 0:M, :]
            P1 = P1_t[:][:, 0:M, :]
            P2 = P2_t[:][:, 0:M, :]
            Q1 = Q1_t[:][:, 0:M, :]
            Q2 = Q2_t[:][:, 0:M, :]
            P2h = P2.rearrange("p m (h j) -> p m h j", h=2)
            Q2h = Q2.rearrange("p m (h j) -> p m h j", h=2)
            cpap = cp16_t[:]
            cpsw = bass.AP(tensor=cpap.tensor, offset=cpap.offset + 8,
                           ap=[list(cpap.ap[0]), [16, M], [-8, 2], [1, 8]])

            cb16 = c[:, :, None].to_broadcast(SH16)
            srb16 = sr[:, :, None].to_broadcast(SH16)
            nsrb16 = nsr[:, :, None].to_broadcast(SH16)
            csb = csi[:, :, :, None].to_broadcast([128, M, 2, 8])
            TT = nc.vector.tensor_tensor
            GT = nc.gpsimd.tensor_tensor

            nc.scalar.copy(cp16, Ap16)               # old col p (re;im)
            GT(P1, srb16, Aq16, ALU.mult)            # [sr*Aqre ; sr*Aqim]
            TT(P2h, csb, Aqsw, ALU.mult)             # [si*Aqim ; -si*Aqre]
            TT(Ap16, cb16, Ap16, ALU.mult)
            TT(Ap16, Ap16, P1, ALU.add)
            TT(Ap16, Ap16, P2, ALU.add)
            GT(Q1, nsrb16, cp16, ALU.mult)           # [-sr*cpre ; -sr*cpim]
            GT(Q2h, csb, cpsw, ALU.mult)             # [si*cpim ; -si*cpre]
            TT(Aq16, cb16, Aq16, ALU.mult)
            TT(Aq16, Aq16, Q1, ALU.add)
            TT(Aq16, Aq16, Q2, ALU.add)
            # Hermitian row restore: row = conj(new col)
            nc.scalar.copy(A4[:, 0:M, p, 0:8], A4[:, 0:M, 0:8, p])
            nc.scalar.activation(A4[:, 0:M, 8 + p, 0:8], A4[:, 0:M, 8:16, p], ACT.Copy, scale=-1.0)
            nc.scalar.copy(A4[:, 0:M, q, 0:8], A4[:, 0:M, 0:8, q])
            nc.scalar.activation(A4[:, 0:M, 8 + q, 0:8], A4[:, 0:M, 8:16, q], ACT.Copy, scale=-1.0)
            # diagonal + annihilated entries
            nc.gpsimd.tensor_copy(A4[:, 0:M, p, p], dpp)
            nc.gpsimd.tensor_copy(A4[:, 0:M, q, q], dqq)
            nc.gpsimd.memset(A4[:, 0:M, 8 + p, p], 0.0)
            nc.gpsimd.memset(A4[:, 0:M, 8 + q, q], 0.0)
            nc.scalar.memzero(A4[:, 0:M, p, q])
            nc.scalar.memzero(A4[:, 0:M, 8 + p, q])
            nc.scalar.memzero(A4[:, 0:M, q, p])
            nc.scalar.memzero(A4[:, 0:M, 8 + q, p])

        for s in range(N_SWEEPS):
            M = NM if s < N_SWEEPS - 1 else NTILES   # last sweep: rho only
            for r in range(1, 8):
                for (p, q) in _xor_pairs(r):
                    emit_rotation(p, q, M)

        # ---- rho diagonal sort (matrices m in [0, NTILES)) ----
        tmin = main.tile([128, NTILES], f32, name="tmin")[:]
        for (i, j) in _CE8:
            di = A4[:, 0:NTILES, i, i]
            dj = A4[:, 0:NTILES, j, j]
            nc.vector.tensor_tensor(tmin, di, dj, ALU.min)
            nc.vector.tensor_tensor(dj, di, dj, ALU.max)
            nc.gpsimd.tensor_copy(di, tmin)

        # ---- pt_a / pt_c diagonal min/max (m in [NTILES, 3*NTILES)) ----
        dv = main.tile([128, 2 * NTILES, 8], f32, name="dv")
        for k in range(8):
            nc.gpsimd.tensor_copy(dv[:, :, k], A4[:, NTILES:NM, k, k])
        mn = main.tile([128, 2 * NTILES], f32, name="mn")[:]
        mx = main.tile([128, 2 * NTILES], f32, name="mx")[:]
        nc.vector.tensor_reduce(mn, dv[:], mybir.AxisListType.X, ALU.min)
        nc.vector.tensor_reduce(mx, dv[:], mybir.AxisListType.X, ALU.max)
        mu_min = mn[:, 0:NTILES]
        mu_max = mx[:, 0:NTILES]
        nu_min = mn[:, NTILES:2 * NTILES]
        nu_max = mx[:, NTILES:2 * NTILES]

        # ---- loss assembly ----
        def L(name):
            return main.tile([128, NTILES], f32, tag=name, name=name)[:]

        w_min = A4[:, 0:NTILES, 0, 0]
        w_max = A4[:, 0:NTILES, 7, 7]
        b0, b1, acc, t1, t2_, t3 = L("b0"), L("b1"), L("acc"), L("t1"), L("t2"), L("t3")

        nc.vector.tensor_scalar(b0, w_min, -8.0, 1.0, ALU.mult, ALU.add)
        nc.vector.reciprocal(b0, b0)
        nc.vector.tensor_scalar(b1, w_max, -8.0, 1.0, ALU.mult, ALU.add)
        nc.vector.reciprocal(b1, b1)

        # S_k0 = sum of k0 smallest, T_k1 = sum of k1 largest
        assert 1 <= k0 <= 8 and 1 <= k1 <= 8
        nc.gpsimd.tensor_copy(t1, A4[:, 0:NTILES, 0, 0])
        for i in range(1, k0):
            nc.vector.tensor_tensor(t1, t1, A4[:, 0:NTILES, i, i], ALU.add)
        nc.gpsimd.tensor_copy(t2_, A4[:, 0:NTILES, 7, 7])
        for i in range(6, 7 - k1, -1):
            nc.vector.tensor_tensor(t2_, t2_, A4[:, 0:NTILES, i, i], ALU.add)
        # loss0 = b0*(S_k0 - k0/8) + k0/8 ; loss1 = b1*(T_k1 - k1/8) + k1/8
        nc.vector.tensor_scalar(t1, t1, -k0 / 8.0, None, ALU.add)
        nc.vector.tensor_tensor(t1, t1, b0, ALU.mult)
        nc.vector.tensor_scalar(t2_, t2_, -k1 / 8.0, None, ALU.add)
        nc.vector.tensor_tensor(t2_, t2_, b1, ALU.mult)
        nc.vector.tensor_tensor(t1, t1, t2_, ALU.add)
        nc.vector.tensor_scalar(t1, t1, (k0 + k1) / 8.0, None, ALU.add)  # l01
        nc.vector.tensor_tensor(acc, t1, t1, ALU.mult)
        for beta, ext in ((b0, mu_min), (b1, mu_max), (b0, nu_min), (b1, nu_max)):
            nc.vector.tensor_scalar(t3, ext, -0.125, None, ALU.add)
            nc.vector.tensor_tensor(t3, t3, beta, ALU.mult)
            nc.vector.tensor_scalar(t3, t3, 0.125, None, ALU.add)
            nc.vector.tensor_tensor(t3, t3, t3, ALU.mult)
            nc.vector.tensor_tensor(acc, acc, t3, ALU.add)

        nc.sync.dma_start(out=out_d[:, :], in_=acc)

    nc.finalize()
    return nc


_prog_cache = {}


def kernel(rho_vec, rank0, rank1):
    rho_vec = np.asarray(rho_vec, dtype=np.float32)
    k0 = D - int(rank0)
    k1 = D - int(rank1)
    in_arrs = _host_prep(rho_vec)

    from concourse.bass_utils import run_bass_kernel_spmd
    key = (k0, k1)
    if key not in _prog_cache:
        _prog_cache[key] = _build_program(k0, k1)
    nc = _prog_cache[key]
    res = run_bass_kernel_spmd(
        nc, [{"mats": a} for a in in_arrs], core_ids=list(range(NCORES)))
    return np.concatenate(
        [np.asarray(res.results[c]["out"]).T.reshape(-1) for c in range(NCORES)]
    ).astype(np.float32)



# revision 12
# speedup vs baseline: 1.6748x; 1.6748x over previous
"""Trainium2 Bass kernel for nn_BESNumEigen3qubitModel (v2).

Math reduction (exact): every eigvalsh in the reference reduces to
eigenvalues of 3 Hermitian 8x8 matrices per batch element: rho, pt_a(rho),
pt_c(rho) (see kernel_baseline for the derivation).

Device algorithm (per core: 4096 batch elems -> 128 partitions x 32 tiles,
3 matrix types -> 96 matrices per partition):
  - Matrix data fp16, layout [128, h(2), i(8), j(8), m(96)] with the matrix
    index m LAST (stride 1) so every DVE operand is packed 2-byte -> 2x/4x
    DVE throughput. Authoritative diagonal kept in f32 [128, 8, 96].
  - Cyclic complex Jacobi in XOR-pair rounds: per round the 4 pairs' rotation
    params are computed batched (exact: a round's 2x2 blocks are disjoint),
    then per-pair column update + Hermitian row restore sequentially.
  - 2 full sweeps (all 96), 1 extra rho-only sweep, then a 2nd-order
    perturbative diag correction for rho from the residual off-diagonal.
    PT matrices only need extremal eigenvalues (their final-round column
    updates are dead and skipped).
  - Sort rho diag (Batcher network), min/max-reduce PT diags, assemble loss.
"""

import numpy as np

D = 8
BATCH = 32768
NCORES = 8
PER_CORE = BATCH // NCORES       # 4096
NTILES = PER_CORE // 128         # 32 tiles per core
NM = 3 * NTILES                  # 96 matrices per partition (type-major)
MR = NM                          # full-round matrix count
MRHO = NTILES                    # rho-only count

# elem strides inside the fp16 matrix tile [2(h), 8(i), 8(j), NM(m)]
SM, SJ, SI, SH = 1, NM, 8 * NM, 64 * NM
ASIZE = 2 * 8 * 8 * NM           # 12288
PDELTA = 1e-6                    # perturbative-correction regularizer

_f32 = np.float32


# ---------------------------------------------------------------- host prep --

def _gellmann_basis(d):
    mats = []
    for j in range(d):
        for k in range(j + 1, d):
            m = np.zeros((d, d), np.complex128); m[j, k] = 1; m[k, j] = 1
            mats.append(m)
    for j in range(d):
        for k in range(j + 1, d):
            m = np.zeros((d, d), np.complex128); m[j, k] = -1j; m[k, j] = 1j
            mats.append(m)
    for l in range(1, d):
        m = np.zeros((d, d), np.complex128)
        m[np.arange(l), np.arange(l)] = 1
        m[l, l] = -l
        mats.append(np.sqrt(2.0 / (l * (l + 1))) * m)
    return np.stack(mats)


def _entry_perm(kind):
    p = np.zeros(64, np.int64)
    for i in range(8):
        for j in range(8):
            if kind == 'a':
                i2, j2 = (j & 4) | (i & 3), (i & 4) | (j & 3)
            else:
                i2, j2 = (i & 6) | (j & 1), (j & 6) | (i & 1)
            p[i * 8 + j] = i2 * 8 + j2
    return p


def _build_maps():
    """[64, 384] f32: (vec,1) -> 128 floats (f = h*64 + i*8 + j) of each of
    rho, pt_a(rho), pt_c(rho)."""
    G = _gellmann_basis(D)
    B = np.zeros((64, 128), np.float64)
    for k in range(63):
        B[k, :64] = G[k].real.reshape(-1)
        B[k, 64:] = G[k].imag.reshape(-1)
    B[63, :64] = (np.eye(D) / D).reshape(-1)

    def float_perm(kind):
        e = _entry_perm(kind)
        return np.concatenate([e, 64 + e])

    M3 = np.concatenate([B, B[:, float_perm('a')], B[:, float_perm('c')]], axis=1)
    return M3.astype(_f32)


_M3 = None


def _host_prep(rho_vec):
    global _M3
    if _M3 is None:
        _M3 = _build_maps()
    vec = rho_vec.astype(np.float64)
    vec = vec / np.linalg.norm(vec, axis=-1, keepdims=True)
    vec_aug = np.concatenate(
        [vec.astype(_f32), np.ones((vec.shape[0], 1), _f32)], axis=1)
    flat = vec_aug @ _M3                                   # [B, 384] f32
    arr = flat.reshape(NCORES, NTILES, 128, 3, 128)        # [core,tile,part,type,f]
    ins = []
    diag_f = np.array([i * 8 + i for i in range(8)])
    for c in range(NCORES):
        a = arr[c]
        # fp16 matrices: [part, f, type, tile] -> [128, f*96 + type*32 + tile]
        m16 = np.ascontiguousarray(
            a.transpose(1, 3, 2, 0).reshape(128, 128 * NM)).astype(np.float16)
        # f32 diag: [part, i, type, tile] -> [128, i*96 + m]
        dg = np.ascontiguousarray(
            a[:, :, :, diag_f].transpose(1, 3, 2, 0).reshape(128, 8 * NM)
        ).astype(_f32)
        ins.append({"mats": m16, "diag": dg})
    return ins


# ------------------------------------------------------------ device kernel --

def _xor_pairs(r):
    return [(i, i ^ r) for i in range(8) if i < (i ^ r)]


def _enum_bits(r):
    """Enumeration bit-steps (descending) for pset = {p: bit_bmax(r)(p)=0},
    enumerated in ascending-p order."""
    bmax = 4 if r >= 4 else (2 if r >= 2 else 1)
    return [b for b in (4, 2, 1) if b != bmax]


# Batcher odd-even mergesort network for 8 elements (19 comparators)
_CE8 = [(0, 1), (2, 3), (4, 5), (6, 7), (0, 2), (1, 3), (4, 6), (5, 7),
        (1, 2), (5, 6), (0, 4), (1, 5), (2, 6), (3, 7), (2, 4), (3, 5),
        (1, 2), (3, 4), (5, 6)]

N_FULL = 2      # full sweeps (all 3 matrix types)
N_RHO = 1       # extra rho-only sweeps


def _build_program(k0, k1):
    import concourse.bass as bass
    import concourse.bacc as bacc
    import concourse.mybir as mybir
    from concourse.tile import TileContext
    from contextlib import ExitStack

    f32 = mybir.dt.float32
    f16 = mybir.dt.float16
    ALU = mybir.AluOpType
    ACT = mybir.ActivationFunctionType

    nc = bacc.Bacc("TRN2")
    mats_d = nc.dram_tensor("mats", [128, ASIZE], f16, kind="ExternalInput")
    diag_d = nc.dram_tensor("diag", [128, 8 * NM], f32, kind="ExternalInput")
    out_d = nc.dram_tensor("out", [128, NTILES], f32, kind="ExternalOutput")

    with ExitStack() as ctx:
        tc = ctx.enter_context(TileContext(nc))
        main = ctx.enter_context(tc.tile_pool(name="main", bufs=1))
        pp = ctx.enter_context(tc.tile_pool(name="pp", bufs=2))
        cp = ctx.enter_context(tc.tile_pool(name="cp", bufs=2))

        A = main.tile([128, ASIZE], f16, name="A")
        Dg = main.tile([128, 8 * NM], f32, name="Dg")
        Aap = A[:]
        Dap = Dg[:]
        pdim = list(Aap.ap[0])

        def av(offset, dims):
            return bass.AP(tensor=Aap.tensor, offset=Aap.offset + offset,
                           ap=[pdim] + dims)

        def dv(offset, dims):
            return bass.AP(tensor=Dap.tensor, offset=Dap.offset + offset,
                           ap=[list(Dap.ap[0])] + dims)

        NCHUNK = 4
        for ch in range(NCHUNK):
            w = ASIZE // NCHUNK
            nc.sync.dma_start(out=av(ch * w, [[1, w]]),
                              in_=mats_d[:, ch * w:(ch + 1) * w])
        nc.sync.dma_start(out=Dg[:], in_=diag_d[:, :])

        eps30 = main.tile([128, 1], f32, name="eps30")
        nc.vector.memset(eps30[:], 1e-30)
        eps35 = main.tile([128, 1], f32, name="eps35")
        nc.vector.memset(eps35[:], 1e-35)

        with nc.allow_low_precision(reason="fp16 Jacobi data by design"):
            _emit_jacobi(nc, bass, mybir, main, pp, cp, av, dv,
                         eps30, eps35, out_d, k0, k1)

    nc.finalize()
    return nc


def _emit_jacobi(nc, bass, mybir, main, pp, cp, av, dv, eps30, eps35,
                 out_d, k0, k1):
    f32 = mybir.dt.float32
    f16 = mybir.dt.float16
    ALU = mybir.AluOpType
    ACT = mybir.ActivationFunctionType
    TT = nc.vector.tensor_tensor
    GT = nc.gpsimd.tensor_tensor
    STT = nc.vector.scalar_tensor_tensor

    def emit_round(r, mp, mu):
        """One Jacobi round: pairs (p, p^r); params on m in [0, mp),
        column updates on m in [0, mu) (mu <= mp, mu == 0 -> params only)."""
        pairs = _xor_pairs(r)
        b1, b2 = _enum_bits(r)  # descending

        def merged(dims):
            if dims[0][0] == 2 * dims[1][0]:
                return [[dims[1][0], 4]] + dims[2:]
            return dims

        sgn = lambda b: -1 if (r & b) else 1
        # X/Y = A[h, p, q] entries: offset q0*SJ (p0=0, q0=r); per-bit step
        # for (p*SI + q*SJ) is b*SI + sgn(b)*b*SJ
        xdims = merged([[b1 * SI + sgn(b1) * b1 * SJ, 2],
                        [b2 * SI + sgn(b2) * b2 * SJ, 2], [1, mp]])
        Xv = av(r * SJ, list(xdims))
        Yv = av(SH + r * SJ, list(xdims))
        # app/aqq from f32 diag (Dg stride per i is NM)
        appv = dv(0, merged([[b1 * NM, 2], [b2 * NM, 2], [1, mp]]))
        aqqv = dv(r * NM, merged([[sgn(b1) * b1 * NM, 2],
                                  [sgn(b2) * b2 * NM, 2], [1, mp]]))

        def P(tag, dt=f32):
            return pp.tile([128, 4, mp], dt, tag=f"{tag}{mp}", name=tag)[:]

        sqx, sqy, n2p, g = P("sqx"), P("sqy"), P("n2p"), P("g")
        gsq, s2, h, ag = P("gsq"), P("s2"), P("h"), P("ag")
        den, T, sg, T2 = P("den"), P("T"), P("sg"), P("T2")
        t2, cden, u, urb2 = P("t2"), P("cden"), P("u"), P("urb2")
        v1, tb = P("v1"), P("tb")
        c16 = P("c16", f16)
        sr16 = P("sr16", f16)
        s2c = pp.tile([128, 2, 4, mp], f16, tag=f"s2c{mp}", name="s2c")[:]

        nc.scalar.activation(sqx, Xv, ACT.Square, scale=2.0)   # 4X^2
        nc.scalar.activation(sqy, Yv, ACT.Square, scale=2.0)
        TT(n2p, sqx, sqy, ALU.add)                             # b'^2
        TT(g, appv, aqqv, ALU.subtract)
        nc.scalar.activation(gsq, g, ACT.Square)
        TT(s2, gsq, n2p, ALU.add)
        nc.scalar.activation(h, s2, ACT.Sqrt, bias=eps30[:])
        nc.scalar.activation(ag, g, ACT.Abs)
        GT(den, ag, h, ALU.add)
        nc.vector.reciprocal(T, den)                           # 1/(|g|+h)
        nc.scalar.sign(sg, g, bias=eps35[:])
        GT(T2, T, T, ALU.mult)
        TT(t2, n2p, T2, ALU.mult)                              # t^2
        nc.scalar.activation(cden, t2, ACT.Sqrt, bias=1.0)
        nc.vector.reciprocal(c16, cden)                        # cos (fp16)
        GT(u, T, sg, ALU.mult)
        STT(urb2, u, 2.0, c16, ALU.mult, ALU.mult)             # 2*t*cos/b'
        TT(sr16, urb2, Xv, ALU.mult)                           # fp16
        TT(s2c[:, 0], urb2, Yv, ALU.mult)                      # +si
        nc.vector.tensor_scalar(s2c[:, 1], s2c[:, 0], -1.0, None, ALU.mult)
        GT(v1, T, n2p, ALU.mult)
        STT(tb, v1, 0.5, sg, ALU.mult, ALU.mult)
        # f32 diag update in place: app += tb, aqq -= tb
        GT(appv, appv, tb, ALU.add)
        GT(aqqv, aqqv, tb, ALU.subtract)

        if mu == 0:
            return

        UD = [[SH, 2], [SI, 8], [1, mu]]
        UDsw = [[-SH, 2], [SI, 8], [1, mu]]

        def CW(tag):
            return cp.tile([128, 2, 8, mu], f16, tag=f"{tag}{mu}", name=tag)[:]

        for k, (p, q) in enumerate(pairs):
            colp = av(p * SJ, list(UD))
            colq = av(q * SJ, list(UD))
            colp_sw = av(SH + p * SJ, list(UDsw))
            colq_sw = av(SH + q * SJ, list(UDsw))
            cb = bass.AP(tensor=c16.tensor, offset=c16.offset + k * mp,
                         ap=[list(c16.ap[0]), [0, 2], [0, 8], [1, mu]])
            srb = bass.AP(tensor=sr16.tensor, offset=sr16.offset + k * mp,
                          ap=[list(sr16.ap[0]), [0, 2], [0, 8], [1, mu]])
            s2b = bass.AP(tensor=s2c.tensor, offset=s2c.offset + k * mp,
                          ap=[list(s2c.ap[0]), [4 * mp, 2], [0, 8], [1, mu]])

            tP, uP, tQ, uQ = CW("tP"), CW("uP"), CW("tQ"), CW("uQ")
            TT(tP, srb, colq, ALU.mult)
            TT(uP, s2b, colq_sw, ALU.mult)
            GT(tQ, srb, colp, ALU.mult)
            TT(uQ, s2b, colp_sw, ALU.mult)
            TT(colp, cb, colp, ALU.mult)
            TT(colp, colp, tP, ALU.add)
            TT(colp, colp, uP, ALU.add)
            if k < 3:
                GT(colq, cb, colq, ALU.mult)
            else:
                TT(colq, cb, colq, ALU.mult)
            TT(colq, colq, tQ, ALU.subtract)
            TT(colq, colq, uQ, ALU.add)

            # Hermitian row restore: rows p,q <- conj(cols p,q)
            ROW = [[SJ, 8], [1, mu]]
            COL = [[SI, 8], [1, mu]]
            nc.scalar.copy(av(p * SI, list(ROW)), av(p * SJ, list(COL)))
            nc.scalar.activation(av(SH + p * SI, list(ROW)),
                                 av(SH + p * SJ, list(COL)), ACT.Copy, scale=-1.0)
            nc.scalar.copy(av(q * SI, list(ROW)), av(q * SJ, list(COL)))
            nc.scalar.activation(av(SH + q * SI, list(ROW)),
                                 av(SH + q * SJ, list(COL)), ACT.Copy, scale=-1.0)

            # diag mirror (fp16 <- f32 Dg) + annihilated-entry zeros
            mdst = av(p * (SI + SJ), [[(q - p) * (SI + SJ), 2], [1, mu]])
            msrc = dv(p * NM, [[(q - p) * NM, 2], [1, mu]])
            nc.scalar.copy(mdst, msrc)
            nc.gpsimd.memset(av(SH + p * (SI + SJ),
                                [[(q - p) * SI, 2], [(q - p) * SJ, 2], [1, mu]]), 0.0)
            nc.gpsimd.memset(av(p * SI + q * SJ,
                                [[(q - p) * (SI - SJ), 2], [1, mu]]), 0.0)

    # ---- sweeps ----
    for s in range(N_FULL):
        for r in range(1, 8):
            last_pt = (s == N_FULL - 1) and r == 7
            emit_round(r, NM, MRHO if last_pt else NM)
    for s in range(N_RHO):
        for r in range(1, 8):
            emit_round(r, MRHO, MRHO)

    # ---- perturbative rho diag correction ----
    M = MRHO

    def Q(tag, dt=f32):
        return main.tile([128, 8, 8, M], dt, tag=tag, name=tag)[:]

    SQ, S, dif, dif2 = Q("pSQ"), Q("pS"), Q("pdif"), Q("pdif2")
    R_, W = Q("pR"), Q("pW")
    corr = main.tile([128, 8, M], f32, tag="pcorr", name="pcorr")[:]
    # |a_ij|^2 in fp32 from fp16 halves
    nc.scalar.activation(SQ, av(0, [[SI, 8], [SJ, 8], [1, M]]), ACT.Square)
    nc.scalar.activation(S, av(SH, [[SI, 8], [SJ, 8], [1, M]]), ACT.Square)
    TT(S, S, SQ, ALU.add)
    TT(dif, dv(0, [[NM, 8], [0, 8], [1, M]]),
       dv(0, [[0, 8], [NM, 8], [1, M]]), ALU.subtract)         # d_i - d_j
    nc.scalar.activation(dif2, dif, ACT.Square)
    dif2_flat = bass.AP(tensor=dif2.tensor, offset=dif2.offset,
                        ap=[list(dif2.ap[0]), [1, 64 * M]])
    nc.vector.tensor_scalar(dif2_flat, dif2_flat, PDELTA, None, ALU.add)
    nc.vector.reciprocal(R_, dif2)
    GT(W, S, dif, ALU.mult)
    TT(W, W, R_, ALU.mult)
    # corr_i = sum_j W[i, j, m]: reduce over j (view with j last)
    Wv = bass.AP(tensor=W.tensor, offset=W.offset,
                 ap=[list(W.ap[0]), [8 * M, 8], [1, M], [M, 8]])
    nc.vector.tensor_reduce(corr, Wv, mybir.AxisListType.X, ALU.add)
    TT(dv(0, [[NM, 8], [1, M]]), dv(0, [[NM, 8], [1, M]]), corr, ALU.add)

    # ---- rho diag sort (ascending) ----
    tmin = main.tile([128, MRHO], f32, name="tmin")[:]
    for (i, j) in _CE8:
        di = dv(i * NM, [[1, MRHO]])
        dj = dv(j * NM, [[1, MRHO]])
        TT(tmin, di, dj, ALU.min)
        TT(dj, di, dj, ALU.max)
        nc.gpsimd.tensor_copy(di, tmin)

    # ---- pt_a / pt_c diag min/max over i ----
    mn = main.tile([128, 2 * NTILES], f32, name="mn")[:]
    mx = main.tile([128, 2 * NTILES], f32, name="mx")[:]
    ptv = dv(NTILES, [[1, 2 * NTILES], [NM, 8]])
    nc.vector.tensor_reduce(mn, ptv, mybir.AxisListType.X, ALU.min)
    nc.vector.tensor_reduce(mx, ptv, mybir.AxisListType.X, ALU.max)
    mu_min = mn[:, 0:NTILES]
    mu_max = mx[:, 0:NTILES]
    nu_min = mn[:, NTILES:2 * NTILES]
    nu_max = mx[:, NTILES:2 * NTILES]

    # ---- loss assembly ----
    def L(name):
        return main.tile([128, NTILES], f32, tag=name, name=name)[:]

    w_min = dv(0, [[1, MRHO]])
    w_max = dv(7 * NM, [[1, MRHO]])
    b0, b1, acc, t1, t2_, t3 = L("b0"), L("b1"), L("acc"), L("t1"), L("t2"), L("t3")

    nc.vector.tensor_scalar(b0, w_min, -8.0, 1.0, ALU.mult, ALU.add)
    nc.vector.reciprocal(b0, b0)
    nc.vector.tensor_scalar(b1, w_max, -8.0, 1.0, ALU.mult, ALU.add)
    nc.vector.reciprocal(b1, b1)

    assert 1 <= k0 <= 8 and 1 <= k1 <= 8
    nc.gpsimd.tensor_copy(t1, dv(0, [[1, MRHO]]))
    for i in range(1, k0):
        TT(t1, t1, dv(i * NM, [[1, MRHO]]), ALU.add)
    nc.gpsimd.tensor_copy(t2_, dv(7 * NM, [[1, MRHO]]))
    for i in range(6, 7 - k1, -1):
        TT(t2_, t2_, dv(i * NM, [[1, MRHO]]), ALU.add)
    nc.vector.tensor_scalar(t1, t1, -k0 / 8.0, None, ALU.add)
    TT(t1, t1, b0, ALU.mult)
    nc.vector.tensor_scalar(t2_, t2_, -k1 / 8.0, None, ALU.add)
    TT(t2_, t2_, b1, ALU.mult)
    TT(t1, t1, t2_, ALU.add)
    nc.vector.tensor_scalar(t1, t1, (k0 + k1) / 8.0, None, ALU.add)
    TT(acc, t1, t1, ALU.mult)
    for beta, ext in ((b0, mu_min), (b1, mu_max), (b0, nu_min), (b1, nu_max)):
        nc.vector.tensor_scalar(t3, ext, -0.125, None, ALU.add)
        TT(t3, t3, beta, ALU.mult)
        nc.vector.tensor_scalar(t3, t3, 0.125, None, ALU.add)
        TT(t3, t3, t3, ALU.mult)
        TT(acc, acc, t3, ALU.add)

    nc.sync.dma_start(out=out_d[:, :], in_=acc)


_prog_cache = {}


def kernel(rho_vec, rank0, rank1):
    rho_vec = np.asarray(rho_vec, dtype=np.float32)
    k0 = D - int(rank0)
    k1 = D - int(rank1)
    in_maps = _host_prep(rho_vec)

    from concourse.bass_utils import run_bass_kernel_spmd
    key = (k0, k1)
    if key not in _prog_cache:
        _prog_cache[key] = _build_program(k0, k1)
    nc = _prog_cache[key]
    res = run_bass_kernel_spmd(nc, in_maps, core_ids=list(range(NCORES)))
    return np.concatenate(
        [np.asarray(res.results[c]["out"]).T.reshape(-1) for c in range(NCORES)]
    ).astype(np.float32)


# revision 13
# speedup vs baseline: 2.2094x; 1.3192x over previous
"""Trainium2 Bass kernel for nn_BESNumEigen3qubitModel (v2).

Math reduction (exact): every eigvalsh in the reference reduces to
eigenvalues of 3 Hermitian 8x8 matrices per batch element: rho, pt_a(rho),
pt_c(rho) (see kernel_baseline for the derivation).

Device algorithm (per core: 4096 batch elems -> 128 partitions x 32 tiles,
3 matrix types -> 96 matrices per partition):
  - Matrix data fp16, layout [128, h(2), i(8), j(8), m(96)] with the matrix
    index m LAST (stride 1) so every DVE operand is packed 2-byte -> 2x/4x
    DVE throughput. Authoritative diagonal kept in f32 [128, 8, 96].
  - Cyclic complex Jacobi in XOR-pair rounds: per round the 4 pairs' rotation
    params are computed batched (exact: a round's 2x2 blocks are disjoint),
    then per-pair column update + Hermitian row restore sequentially.
  - 2 full sweeps (all 96), 1 extra rho-only sweep, then a 2nd-order
    perturbative diag correction for rho from the residual off-diagonal.
    PT matrices only need extremal eigenvalues (their final-round column
    updates are dead and skipped).
  - Sort rho diag (Batcher network), min/max-reduce PT diags, assemble loss.
"""

import numpy as np

D = 8
BATCH = 32768
NCORES = 8
PER_CORE = BATCH // NCORES       # 4096
NTILES = PER_CORE // 128         # 32 tiles per core
NM = 3 * NTILES                  # 96 matrices per partition (type-major)
MR = NM                          # full-round matrix count
MRHO = NTILES                    # rho-only count

# elem strides inside the fp16 matrix tile [2(h), 8(i), 8(j), NM(m)]
SM, SJ, SI, SH = 1, NM, 8 * NM, 64 * NM
ASIZE = 2 * 8 * 8 * NM           # 12288
PDELTA = 1e-6                    # perturbative-correction regularizer

_f32 = np.float32


# ---------------------------------------------------------------- host prep --

def _gellmann_basis(d):
    mats = []
    for j in range(d):
        for k in range(j + 1, d):
            m = np.zeros((d, d), np.complex128); m[j, k] = 1; m[k, j] = 1
            mats.append(m)
    for j in range(d):
        for k in range(j + 1, d):
            m = np.zeros((d, d), np.complex128); m[j, k] = -1j; m[k, j] = 1j
            mats.append(m)
    for l in range(1, d):
        m = np.zeros((d, d), np.complex128)
        m[np.arange(l), np.arange(l)] = 1
        m[l, l] = -l
        mats.append(np.sqrt(2.0 / (l * (l + 1))) * m)
    return np.stack(mats)


def _entry_perm(kind):
    p = np.zeros(64, np.int64)
    for i in range(8):
        for j in range(8):
            if kind == 'a':
                i2, j2 = (j & 4) | (i & 3), (i & 4) | (j & 3)
            else:
                i2, j2 = (i & 6) | (j & 1), (j & 6) | (i & 1)
            p[i * 8 + j] = i2 * 8 + j2
    return p


def _build_maps():
    """[64, 384] f32: (vec,1) -> 128 floats (f = h*64 + i*8 + j) of each of
    rho, pt_a(rho), pt_c(rho)."""
    G = _gellmann_basis(D)
    B = np.zeros((64, 128), np.float64)
    for k in range(63):
        B[k, :64] = G[k].real.reshape(-1)
        B[k, 64:] = G[k].imag.reshape(-1)
    B[63, :64] = (np.eye(D) / D).reshape(-1)

    def float_perm(kind):
        e = _entry_perm(kind)
        return np.concatenate([e, 64 + e])

    M3 = np.concatenate([B, B[:, float_perm('a')], B[:, float_perm('c')]], axis=1)
    return M3.astype(_f32)


_M3 = None


def _host_prep(rho_vec):
    global _M3
    if _M3 is None:
        _M3 = _build_maps()
    vec = rho_vec.astype(np.float64)
    vec = vec / np.linalg.norm(vec, axis=-1, keepdims=True)
    vec_aug = np.concatenate(
        [vec.astype(_f32), np.ones((vec.shape[0], 1), _f32)], axis=1)
    flat = vec_aug @ _M3                                   # [B, 384] f32
    arr = flat.reshape(NCORES, NTILES, 128, 3, 128)        # [core,tile,part,type,f]
    ins = []
    diag_f = np.array([i * 8 + i for i in range(8)])
    for c in range(NCORES):
        a = arr[c]
        # fp16 matrices: [part, f, type, tile] -> [128, f*96 + type*32 + tile]
        m16 = np.ascontiguousarray(
            a.transpose(1, 3, 2, 0).reshape(128, 128 * NM)).astype(np.float16)
        # f32 diag: [part, i, type, tile] -> [128, i*96 + m]
        dg = np.ascontiguousarray(
            a[:, :, :, diag_f].transpose(1, 3, 2, 0).reshape(128, 8 * NM)
        ).astype(_f32)
        ins.append({"mats": m16, "diag": dg})
    return ins


# ------------------------------------------------------------ device kernel --

def _xor_pairs(r):
    return [(i, i ^ r) for i in range(8) if i < (i ^ r)]


def _enum_bits(r):
    """Enumeration bit-steps (descending) for pset = {p: bit_bmax(r)(p)=0},
    enumerated in ascending-p order."""
    bmax = 4 if r >= 4 else (2 if r >= 2 else 1)
    return [b for b in (4, 2, 1) if b != bmax]


# Batcher odd-even mergesort network for 8 elements (19 comparators)
_CE8 = [(0, 1), (2, 3), (4, 5), (6, 7), (0, 2), (1, 3), (4, 6), (5, 7),
        (1, 2), (5, 6), (0, 4), (1, 5), (2, 6), (3, 7), (2, 4), (3, 5),
        (1, 2), (3, 4), (5, 6)]

N_FULL = 2      # full sweeps (all 3 matrix types)
N_RHO = 1       # extra rho-only sweeps


def _build_program(k0, k1):
    import concourse.bass as bass
    import concourse.bacc as bacc
    import concourse.mybir as mybir
    from concourse.tile import TileContext
    from contextlib import ExitStack

    f32 = mybir.dt.float32
    f16 = mybir.dt.float16
    ALU = mybir.AluOpType
    ACT = mybir.ActivationFunctionType

    nc = bacc.Bacc("TRN2")
    mats_d = nc.dram_tensor("mats", [128, ASIZE], f16, kind="ExternalInput")
    diag_d = nc.dram_tensor("diag", [128, 8 * NM], f32, kind="ExternalInput")
    out_d = nc.dram_tensor("out", [128, NTILES], f32, kind="ExternalOutput")

    with ExitStack() as ctx:
        tc = ctx.enter_context(TileContext(nc))
        main = ctx.enter_context(tc.tile_pool(name="main", bufs=1))
        pp = ctx.enter_context(tc.tile_pool(name="pp", bufs=2))
        cp = ctx.enter_context(tc.tile_pool(name="cp", bufs=2))

        A = main.tile([128, ASIZE], f16, name="A")
        Dg = main.tile([128, 8 * NM], f32, name="Dg")
        Aap = A[:]
        Dap = Dg[:]
        pdim = list(Aap.ap[0])

        def av(offset, dims):
            return bass.AP(tensor=Aap.tensor, offset=Aap.offset + offset,
                           ap=[pdim] + dims)

        def dv(offset, dims):
            return bass.AP(tensor=Dap.tensor, offset=Dap.offset + offset,
                           ap=[list(Dap.ap[0])] + dims)

        NCHUNK = 4
        for ch in range(NCHUNK):
            w = ASIZE // NCHUNK
            nc.sync.dma_start(out=av(ch * w, [[1, w]]),
                              in_=mats_d[:, ch * w:(ch + 1) * w])
        nc.sync.dma_start(out=Dg[:], in_=diag_d[:, :])

        eps30 = main.tile([128, 1], f32, name="eps30")
        nc.vector.memset(eps30[:], 1e-30)
        eps35 = main.tile([128, 1], f32, name="eps35")
        nc.vector.memset(eps35[:], 1e-35)

        with nc.allow_low_precision(reason="fp16 Jacobi data by design"):
            _emit_jacobi(nc, bass, mybir, main, pp, cp, av, dv,
                         eps30, eps35, out_d, k0, k1)

    nc.finalize()
    return nc


def _emit_jacobi(nc, bass, mybir, main, pp, cp, av, dv, eps30, eps35,
                 out_d, k0, k1):
    f32 = mybir.dt.float32
    f16 = mybir.dt.float16
    ALU = mybir.AluOpType
    ACT = mybir.ActivationFunctionType
    TT = nc.vector.tensor_tensor
    GT = nc.gpsimd.tensor_tensor
    STT = nc.vector.scalar_tensor_tensor

    neg1 = main.tile([128, NM], f16, name="neg1")
    nc.vector.memset(neg1[:], -1.0)
    n1ap = neg1[:]

    def emit_params(r, m0, mp):
        """Rotation params for round r, matrices m in [m0, m0+mp).
        Returns (c16, sr16, s2c) coefficient tiles [128, 4, mp] (+[2] for s2c)."""
        b1, b2 = _enum_bits(r)  # descending

        def merged(dims):
            if dims[0][0] == 2 * dims[1][0]:
                return [[dims[1][0], 4]] + dims[2:]
            return dims

        sgn = lambda b: -1 if (r & b) else 1
        xdims = merged([[b1 * SI + sgn(b1) * b1 * SJ, 2],
                        [b2 * SI + sgn(b2) * b2 * SJ, 2], [1, mp]])
        Xv = av(r * SJ + m0, list(xdims))
        Yv = av(SH + r * SJ + m0, list(xdims))
        appv = dv(m0, merged([[b1 * NM, 2], [b2 * NM, 2], [1, mp]]))
        aqqv = dv(r * NM + m0, merged([[sgn(b1) * b1 * NM, 2],
                                       [sgn(b2) * b2 * NM, 2], [1, mp]]))

        def P(tag, dt=f32):
            return pp.tile([128, 4, mp], dt, tag=f"{tag}{mp}g{m0}", name=tag)[:]

        sqx, sqy, n2p, g = P("sqx"), P("sqy"), P("n2p"), P("g")
        gsq, s2, h, ag = P("gsq"), P("s2"), P("h"), P("ag")
        den, T, sg, T2 = P("den"), P("T"), P("sg"), P("T2")
        t2, cden, u, urb2 = P("t2"), P("cden"), P("u"), P("urb2")
        v1, tb = P("v1"), P("tb")
        c16 = P("c16", f16)
        sr16 = P("sr16", f16)
        s2c = pp.tile([128, 2, 4, mp], f16, tag=f"s2c{mp}g{m0}", name="s2c")[:]

        nc.scalar.activation(sqx, Xv, ACT.Square, scale=2.0)   # 4X^2
        nc.scalar.activation(sqy, Yv, ACT.Square, scale=2.0)
        TT(n2p, sqx, sqy, ALU.add)                             # b'^2
        TT(g, appv, aqqv, ALU.subtract)
        nc.scalar.activation(gsq, g, ACT.Square)
        TT(s2, gsq, n2p, ALU.add)
        nc.scalar.activation(h, s2, ACT.Sqrt, bias=eps30[:])
        nc.scalar.activation(ag, g, ACT.Abs)
        GT(den, ag, h, ALU.add)
        nc.vector.reciprocal(T, den)                           # 1/(|g|+h)
        nc.scalar.sign(sg, g, bias=eps35[:])
        GT(T2, T, T, ALU.mult)
        TT(t2, n2p, T2, ALU.mult)                              # t^2
        nc.scalar.activation(cden, t2, ACT.Sqrt, bias=1.0)
        nc.vector.reciprocal(c16, cden)                        # cos (fp16)
        GT(u, T, sg, ALU.mult)
        STT(urb2, u, 2.0, c16, ALU.mult, ALU.mult)             # 2*t*cos/b'
        TT(sr16, urb2, Xv, ALU.mult)                           # fp16
        TT(s2c[:, 0], urb2, Yv, ALU.mult)                      # +si
        nc.vector.tensor_scalar(s2c[:, 1], s2c[:, 0], -1.0, None, ALU.mult)
        GT(v1, T, n2p, ALU.mult)
        STT(tb, v1, 0.5, sg, ALU.mult, ALU.mult)
        # f32 diag update in place: app += tb, aqq -= tb
        GT(appv, appv, tb, ALU.add)
        GT(aqqv, aqqv, tb, ALU.subtract)
        return c16, sr16, s2c

    def emit_pair(k, p, q, m0, mu, mp, coeffs):
        """Column update + Hermitian restore for pair (p, q), m in [m0, m0+mu)."""
        c16, sr16, s2c = coeffs
        UD = [[SH, 2], [SI, 8], [1, mu]]
        UDsw = [[-SH, 2], [SI, 8], [1, mu]]
        colp = av(p * SJ + m0, list(UD))
        colq = av(q * SJ + m0, list(UD))
        colp_sw = av(SH + p * SJ + m0, list(UDsw))
        colq_sw = av(SH + q * SJ + m0, list(UDsw))
        cb = bass.AP(tensor=c16.tensor, offset=c16.offset + k * mp,
                     ap=[list(c16.ap[0]), [0, 2], [0, 8], [1, mu]])
        srb = bass.AP(tensor=sr16.tensor, offset=sr16.offset + k * mp,
                      ap=[list(sr16.ap[0]), [0, 2], [0, 8], [1, mu]])
        s2b = bass.AP(tensor=s2c.tensor, offset=s2c.offset + k * mp,
                      ap=[list(s2c.ap[0]), [4 * mp, 2], [0, 8], [1, mu]])

        def CW(tag):
            return cp.tile([128, 2, 8, mu], f16, tag=f"{tag}{mu}g{m0}",
                           name=tag)[:]

        tP, uP, tQ, uQ = CW("tP"), CW("uP"), CW("tQ"), CW("uQ")
        TT(tP, srb, colq, ALU.mult)
        TT(uP, s2b, colq_sw, ALU.mult)
        GT(tQ, srb, colp, ALU.mult)
        TT(uQ, s2b, colp_sw, ALU.mult)
        TT(colp, cb, colp, ALU.mult)
        TT(colp, colp, tP, ALU.add)
        TT(colp, colp, uP, ALU.add)
        TT(colq, cb, colq, ALU.mult)
        TT(colq, colq, tQ, ALU.subtract)
        TT(colq, colq, uQ, ALU.add)

        # Hermitian row restore (merged rows p,q): rows <- conj(cols).
        # The (p,q)/(q,p) entries race within the merged ops but are
        # explicitly zeroed below.
        dROW = [[(q - p) * SI, 2], [SJ, 8], [1, mu]]
        sCOL = [[(q - p) * SJ, 2], [SI, 8], [1, mu]]
        nc.scalar.copy(av(p * SI + m0, list(dROW)), av(p * SJ + m0, list(sCOL)))
        n1v = bass.AP(tensor=n1ap.tensor, offset=n1ap.offset,
                      ap=[list(n1ap.ap[0]), [0, 2], [0, 8], [1, mu]])
        TT(av(SH + p * SI + m0, list(dROW)), av(SH + p * SJ + m0, list(sCOL)),
           n1v, ALU.mult)

        # diag mirror (fp16 <- f32 Dg) + annihilated-entry zeros
        mdst = av(p * (SI + SJ) + m0, [[(q - p) * (SI + SJ), 2], [1, mu]])
        msrc = dv(p * NM + m0, [[(q - p) * NM, 2], [1, mu]])
        nc.gpsimd.tensor_copy(mdst, msrc)
        nc.scalar.memzero(av(SH + p * (SI + SJ) + m0,
                             [[(q - p) * SI, 2], [(q - p) * SJ, 2], [1, mu]]))
        nc.gpsimd.memset(av(p * SI + q * SJ + m0,
                            [[(q - p) * (SI - SJ), 2], [1, mu]]), 0.0)

    # ---- sweeps ----
    GRP = ((0, 48), (48, 48))          # F-part m-groups
    for s in range(N_FULL):
        for r in range(1, 8):
            last_pt = (s == N_FULL - 1) and r == 7
            pairs = _xor_pairs(r)
            if last_pt:
                co = [emit_params(r, 0, 48), emit_params(r, 48, 48)]
                for k, (p, q) in enumerate(pairs):
                    emit_pair(k, p, q, 0, MRHO, 48, co[0])
            else:
                co = [emit_params(r, m0, mc) for (m0, mc) in GRP]
                for k, (p, q) in enumerate(pairs):
                    for gi, (m0, mc) in enumerate(GRP):
                        emit_pair(k, p, q, m0, mc, mc, co[gi])
    for s in range(N_RHO):
        for r in range(1, 8):
            pairs = _xor_pairs(r)
            co = emit_params(r, 0, MRHO)
            for k, (p, q) in enumerate(pairs):
                emit_pair(k, p, q, 0, MRHO, MRHO, co)

    M = MRHO

    # ---- perturbative rho diag correction ----
    M = MRHO

    def Q(tag, dt=f32):
        return main.tile([128, 8, 8, M], dt, tag=tag, name=tag)[:]

    SQ, S, dif, dif2 = Q("pSQ"), Q("pS"), Q("pdif"), Q("pdif2")
    R_, W = Q("pR"), Q("pW")
    corr = main.tile([128, 8, M], f32, tag="pcorr", name="pcorr")[:]
    # |a_ij|^2 in fp32 from fp16 halves
    nc.scalar.activation(SQ, av(0, [[SI, 8], [SJ, 8], [1, M]]), ACT.Square)
    nc.scalar.activation(S, av(SH, [[SI, 8], [SJ, 8], [1, M]]), ACT.Square)
    TT(S, S, SQ, ALU.add)
    TT(dif, dv(0, [[NM, 8], [0, 8], [1, M]]),
       dv(0, [[0, 8], [NM, 8], [1, M]]), ALU.subtract)         # d_i - d_j
    nc.scalar.activation(dif2, dif, ACT.Square)
    dif2_flat = bass.AP(tensor=dif2.tensor, offset=dif2.offset,
                        ap=[list(dif2.ap[0]), [1, 64 * M]])
    nc.vector.tensor_scalar(dif2_flat, dif2_flat, PDELTA, None, ALU.add)
    nc.vector.reciprocal(R_, dif2)
    GT(W, S, dif, ALU.mult)
    TT(W, W, R_, ALU.mult)
    # corr_i = sum_j W[i, j, m]: reduce over j (view with j last)
    Wv = bass.AP(tensor=W.tensor, offset=W.offset,
                 ap=[list(W.ap[0]), [8 * M, 8], [1, M], [M, 8]])
    nc.vector.tensor_reduce(corr, Wv, mybir.AxisListType.X, ALU.add)
    TT(dv(0, [[NM, 8], [1, M]]), dv(0, [[NM, 8], [1, M]]), corr, ALU.add)

    # ---- rho diag sort (ascending) ----
    tmin = main.tile([128, MRHO], f32, name="tmin")[:]
    for (i, j) in _CE8:
        di = dv(i * NM, [[1, MRHO]])
        dj = dv(j * NM, [[1, MRHO]])
        TT(tmin, di, dj, ALU.min)
        TT(dj, di, dj, ALU.max)
        nc.gpsimd.tensor_copy(di, tmin)

    # ---- pt_a / pt_c diag min/max over i ----
    mn = main.tile([128, 2 * NTILES], f32, name="mn")[:]
    mx = main.tile([128, 2 * NTILES], f32, name="mx")[:]
    ptv = dv(NTILES, [[1, 2 * NTILES], [NM, 8]])
    nc.vector.tensor_reduce(mn, ptv, mybir.AxisListType.X, ALU.min)
    nc.vector.tensor_reduce(mx, ptv, mybir.AxisListType.X, ALU.max)
    mu_min = mn[:, 0:NTILES]
    mu_max = mx[:, 0:NTILES]
    nu_min = mn[:, NTILES:2 * NTILES]
    nu_max = mx[:, NTILES:2 * NTILES]

    # ---- loss assembly ----
    def L(name):
        return main.tile([128, NTILES], f32, tag=name, name=name)[:]

    w_min = dv(0, [[1, MRHO]])
    w_max = dv(7 * NM, [[1, MRHO]])
    b0, b1, acc, t1, t2_, t3 = L("b0"), L("b1"), L("acc"), L("t1"), L("t2"), L("t3")

    nc.vector.tensor_scalar(b0, w_min, -8.0, 1.0, ALU.mult, ALU.add)
    nc.vector.reciprocal(b0, b0)
    nc.vector.tensor_scalar(b1, w_max, -8.0, 1.0, ALU.mult, ALU.add)
    nc.vector.reciprocal(b1, b1)

    assert 1 <= k0 <= 8 and 1 <= k1 <= 8
    nc.gpsimd.tensor_copy(t1, dv(0, [[1, MRHO]]))
    for i in range(1, k0):
        TT(t1, t1, dv(i * NM, [[1, MRHO]]), ALU.add)
    nc.gpsimd.tensor_copy(t2_, dv(7 * NM, [[1, MRHO]]))
    for i in range(6, 7 - k1, -1):
        TT(t2_, t2_, dv(i * NM, [[1, MRHO]]), ALU.add)
    nc.vector.tensor_scalar(t1, t1, -k0 / 8.0, None, ALU.add)
    TT(t1, t1, b0, ALU.mult)
    nc.vector.tensor_scalar(t2_, t2_, -k1 / 8.0, None, ALU.add)
    TT(t2_, t2_, b1, ALU.mult)
    TT(t1, t1, t2_, ALU.add)
    nc.vector.tensor_scalar(t1, t1, (k0 + k1) / 8.0, None, ALU.add)
    TT(acc, t1, t1, ALU.mult)
    for beta, ext in ((b0, mu_min), (b1, mu_max), (b0, nu_min), (b1, nu_max)):
        nc.vector.tensor_scalar(t3, ext, -0.125, None, ALU.add)
        TT(t3, t3, beta, ALU.mult)
        nc.vector.tensor_scalar(t3, t3, 0.125, None, ALU.add)
        TT(t3, t3, t3, ALU.mult)
        TT(acc, acc, t3, ALU.add)

    nc.sync.dma_start(out=out_d[:, :], in_=acc)


_prog_cache = {}


def kernel(rho_vec, rank0, rank1):
    rho_vec = np.asarray(rho_vec, dtype=np.float32)
    k0 = D - int(rank0)
    k1 = D - int(rank1)
    in_maps = _host_prep(rho_vec)

    from concourse.bass_utils import run_bass_kernel_spmd
    key = (k0, k1)
    if key not in _prog_cache:
        _prog_cache[key] = _build_program(k0, k1)
    nc = _prog_cache[key]
    res = run_bass_kernel_spmd(nc, in_maps, core_ids=list(range(NCORES)))
    return np.concatenate(
        [np.asarray(res.results[c]["out"]).T.reshape(-1) for c in range(NCORES)]
    ).astype(np.float32)


# revision 29
# speedup vs baseline: 2.3090x; 1.0451x over previous
"""Trainium2 Bass kernel for nn_BESNumEigen3qubitModel (v2).

Math reduction (exact): every eigvalsh in the reference reduces to
eigenvalues of 3 Hermitian 8x8 matrices per batch element: rho, pt_a(rho),
pt_c(rho) (see kernel_baseline for the derivation).

Device algorithm (per core: 4096 batch elems -> 128 partitions x 32 tiles,
3 matrix types -> 96 matrices per partition):
  - Matrix data fp16, layout [128, h(2), i(8), j(8), m(96)] with the matrix
    index m LAST (stride 1) so every DVE operand is packed 2-byte -> 2x/4x
    DVE throughput. Authoritative diagonal kept in f32 [128, 8, 96].
  - Cyclic complex Jacobi in XOR-pair rounds: per round the 4 pairs' rotation
    params are computed batched (exact: a round's 2x2 blocks are disjoint),
    then per-pair column update + Hermitian row restore sequentially.
  - 2 full sweeps (all 96), 1 extra rho-only sweep, then a 2nd-order
    perturbative diag correction for rho from the residual off-diagonal.
    PT matrices only need extremal eigenvalues (their final-round column
    updates are dead and skipped).
  - Sort rho diag (Batcher network), min/max-reduce PT diags, assemble loss.
"""

import numpy as np

D = 8
BATCH = 32768
NCORES = 8
PER_CORE = BATCH // NCORES       # 4096
NTILES = PER_CORE // 128         # 32 tiles per core
NM = 3 * NTILES                  # 96 matrices per partition (type-major)
MR = NM                          # full-round matrix count
MRHO = NTILES                    # rho-only count

# elem strides inside the fp16 matrix tile [2(h), 8(i), 8(j), NM(m)]
SM, SJ, SI, SH = 1, NM, 8 * NM, 64 * NM
ASIZE = 2 * 8 * 8 * NM           # 12288
PDELTA = 1e-6                    # perturbative-correction regularizer

_f32 = np.float32


# ---------------------------------------------------------------- host prep --

def _gellmann_basis(d):
    mats = []
    for j in range(d):
        for k in range(j + 1, d):
            m = np.zeros((d, d), np.complex128); m[j, k] = 1; m[k, j] = 1
            mats.append(m)
    for j in range(d):
        for k in range(j + 1, d):
            m = np.zeros((d, d), np.complex128); m[j, k] = -1j; m[k, j] = 1j
            mats.append(m)
    for l in range(1, d):
        m = np.zeros((d, d), np.complex128)
        m[np.arange(l), np.arange(l)] = 1
        m[l, l] = -l
        mats.append(np.sqrt(2.0 / (l * (l + 1))) * m)
    return np.stack(mats)


def _entry_perm(kind):
    p = np.zeros(64, np.int64)
    for i in range(8):
        for j in range(8):
            if kind == 'a':
                i2, j2 = (j & 4) | (i & 3), (i & 4) | (j & 3)
            else:
                i2, j2 = (i & 6) | (j & 1), (j & 6) | (i & 1)
            p[i * 8 + j] = i2 * 8 + j2
    return p


def _build_maps():
    """[64, 384] f32: (vec,1) -> 128 floats (f = h*64 + i*8 + j) of each of
    rho, pt_a(rho), pt_c(rho)."""
    G = _gellmann_basis(D)
    B = np.zeros((64, 128), np.float64)
    for k in range(63):
        B[k, :64] = G[k].real.reshape(-1)
        B[k, 64:] = G[k].imag.reshape(-1)
    B[63, :64] = (np.eye(D) / D).reshape(-1)

    def float_perm(kind):
        e = _entry_perm(kind)
        return np.concatenate([e, 64 + e])

    M3 = np.concatenate([B, B[:, float_perm('a')], B[:, float_perm('c')]], axis=1)
    return M3.astype(_f32)


_M3 = None


def _host_prep(rho_vec):
    global _M3
    if _M3 is None:
        _M3 = _build_maps()
    vec = rho_vec.astype(np.float64)
    vec = vec / np.linalg.norm(vec, axis=-1, keepdims=True)
    vec_aug = np.concatenate(
        [vec.astype(_f32), np.ones((vec.shape[0], 1), _f32)], axis=1)
    flat = vec_aug @ _M3                                   # [B, 384] f32
    arr = flat.reshape(NCORES, NTILES, 128, 3, 128)        # [core,tile,part,type,f]
    ins = []
    diag_f = np.array([i * 8 + i for i in range(8)])
    for c in range(NCORES):
        a = arr[c]
        # fp16 matrices: [part, f, type, tile] -> [128, f*96 + type*32 + tile]
        m16 = np.ascontiguousarray(
            a.transpose(1, 3, 2, 0).reshape(128, 128 * NM)).astype(np.float16)
        # f32 diag: [part, i, type, tile] -> [128, i*96 + m]
        dg = np.ascontiguousarray(
            a[:, :, :, diag_f].transpose(1, 3, 2, 0).reshape(128, 8 * NM)
        ).astype(_f32)
        ins.append({"mats": m16, "diag": dg})
    return ins


# ------------------------------------------------------------ device kernel --

def _xor_pairs(r):
    return [(i, i ^ r) for i in range(8) if i < (i ^ r)]


def _enum_bits(r):
    """Enumeration bit-steps (descending) for pset = {p: bit_bmax(r)(p)=0},
    enumerated in ascending-p order."""
    bmax = 4 if r >= 4 else (2 if r >= 2 else 1)
    return [b for b in (4, 2, 1) if b != bmax]


# Batcher odd-even mergesort network for 8 elements (19 comparators)
_CE8 = [(0, 1), (2, 3), (4, 5), (6, 7), (0, 2), (1, 3), (4, 6), (5, 7),
        (1, 2), (5, 6), (0, 4), (1, 5), (2, 6), (3, 7), (2, 4), (3, 5),
        (1, 2), (3, 4), (5, 6)]

N_FULL = 2      # full sweeps (all 3 matrix types)
N_RHO = 1       # extra rho-only sweeps


def _build_program(k0, k1):
    import concourse.bass as bass
    import concourse.bacc as bacc
    import concourse.mybir as mybir
    from concourse.tile import TileContext
    from contextlib import ExitStack

    f32 = mybir.dt.float32
    f16 = mybir.dt.float16
    ALU = mybir.AluOpType
    ACT = mybir.ActivationFunctionType

    nc = bacc.Bacc("TRN2")
    mats_d = nc.dram_tensor("mats", [128, ASIZE], f16, kind="ExternalInput")
    diag_d = nc.dram_tensor("diag", [128, 8 * NM], f32, kind="ExternalInput")
    out_d = nc.dram_tensor("out", [128, NTILES], f32, kind="ExternalOutput")

    with ExitStack() as ctx:
        tc = ctx.enter_context(TileContext(nc))
        main = ctx.enter_context(tc.tile_pool(name="main", bufs=1))
        pp = ctx.enter_context(tc.tile_pool(name="pp", bufs=2))
        cp = ctx.enter_context(tc.tile_pool(name="cp", bufs=2))

        A = main.tile([128, ASIZE], f16, name="A")
        Dg = main.tile([128, 8 * NM], f32, name="Dg")
        Aap = A[:]
        Dap = Dg[:]
        pdim = list(Aap.ap[0])

        def av(offset, dims):
            return bass.AP(tensor=Aap.tensor, offset=Aap.offset + offset,
                           ap=[pdim] + dims)

        def dv(offset, dims):
            return bass.AP(tensor=Dap.tensor, offset=Dap.offset + offset,
                           ap=[list(Dap.ap[0])] + dims)

        nc.sync.dma_start(out=Dg[:], in_=diag_d[:, :])
        NCHUNK = 4
        for ch in range(NCHUNK):
            w = ASIZE // NCHUNK
            nc.sync.dma_start(out=av(ch * w, [[1, w]]),
                              in_=mats_d[:, ch * w:(ch + 1) * w])

        eps30 = main.tile([128, 1], f32, name="eps30")
        nc.vector.memset(eps30[:], 1e-30)
        eps35 = main.tile([128, 1], f32, name="eps35")
        nc.vector.memset(eps35[:], 1e-35)

        with nc.allow_low_precision(reason="fp16 Jacobi data by design"):
            _emit_jacobi(nc, bass, mybir, main, pp, cp, av, dv,
                         eps30, eps35, out_d, k0, k1)

    nc.finalize()
    return nc


def _emit_jacobi(nc, bass, mybir, main, pp, cp, av, dv, eps30, eps35,
                 out_d, k0, k1):
    f32 = mybir.dt.float32
    f16 = mybir.dt.float16
    ALU = mybir.AluOpType
    ACT = mybir.ActivationFunctionType
    TT = nc.vector.tensor_tensor
    GT = nc.gpsimd.tensor_tensor
    STT = nc.vector.scalar_tensor_tensor

    neg1 = main.tile([128, NM], f16, name="neg1")
    nc.vector.memset(neg1[:], -1.0)
    n1ap = neg1[:]

    def make_params(r, m0, mp):
        """Rotation params for round r, matrices m in [m0, m0+mp).
        Returns ((c16, sr16, s2c), [op thunks]) - thunks emit one op each,
        in dependency order, so callers can interleave them with other work."""
        b1, b2 = _enum_bits(r)  # descending

        def merged(dims):
            if dims[0][0] == 2 * dims[1][0]:
                return [[dims[1][0], 4]] + dims[2:]
            return dims

        sgn = lambda b: -1 if (r & b) else 1
        xdims = merged([[b1 * SI + sgn(b1) * b1 * SJ, 2],
                        [b2 * SI + sgn(b2) * b2 * SJ, 2], [1, mp]])
        Xv = av(r * SJ + m0, list(xdims))
        Yv = av(SH + r * SJ + m0, list(xdims))
        appv = dv(m0, merged([[b1 * NM, 2], [b2 * NM, 2], [1, mp]]))
        aqqv = dv(r * NM + m0, merged([[sgn(b1) * b1 * NM, 2],
                                       [sgn(b2) * b2 * NM, 2], [1, mp]]))

        def P(tag, dt=f32):
            return pp.tile([128, 4, mp], dt, tag=f"{tag}{mp}g{m0}", name=tag)[:]

        sqx, sqy, n2p, g = P("sqx"), P("sqy"), P("n2p"), P("g")
        gsq, s2, h, ag = P("gsq"), P("s2"), P("h"), P("ag")
        den, T, sg, T2 = P("den"), P("T"), P("sg"), P("T2")
        t2, cden, u, urb2 = P("t2"), P("cden"), P("u"), P("urb2")
        tb = P("tb")
        c16 = P("c16", f16)
        sr16 = P("sr16", f16)
        s2c = pp.tile([128, 2, 4, mp], f16, tag=f"s2c{mp}g{m0}", name="s2c")[:]

        ops = [
            lambda: nc.scalar.activation(sqx, Xv, ACT.Square, scale=2.0),
            lambda: nc.scalar.activation(sqy, Yv, ACT.Square, scale=2.0),
            lambda: TT(g, appv, aqqv, ALU.subtract),
            lambda: nc.scalar.activation(ag, g, ACT.Abs),
            lambda: nc.scalar.sign(sg, g, bias=eps35[:]),
            lambda: nc.scalar.activation(gsq, g, ACT.Square),
            lambda: TT(n2p, sqx, sqy, ALU.add),
            lambda: TT(s2, gsq, n2p, ALU.add),
            lambda: nc.scalar.activation(h, s2, ACT.Sqrt, bias=eps30[:]),
            lambda: GT(den, ag, h, ALU.add),
            lambda: nc.vector.reciprocal(T, den),
            lambda: GT(T2, T, T, ALU.mult),
            lambda: TT(t2, n2p, T2, ALU.mult),
            lambda: nc.scalar.activation(cden, t2, ACT.Sqrt, bias=1.0),
            lambda: nc.vector.reciprocal(c16, cden),
            lambda: GT(u, T, sg, ALU.mult),
            lambda: STT(urb2, u, 2.0, c16, ALU.mult, ALU.mult),
            lambda: TT(sr16, urb2, Xv, ALU.mult),
            lambda: TT(s2c[:, 0], urb2, Yv, ALU.mult),
            lambda: nc.vector.tensor_scalar(s2c[:, 1], s2c[:, 0], -1.0,
                                            None, ALU.mult),
            lambda: STT(tb, u, 0.5, n2p, ALU.mult, ALU.mult),
            lambda: GT(appv, appv, tb, ALU.add),
            lambda: GT(aqqv, aqqv, tb, ALU.subtract),
        ]
        return (c16, sr16, s2c), ops

    def emit_pair(k, p, q, m0, mu, mp, coeffs, cofs=0):
        """Column update + Hermitian restore for pair (p, q), m in [m0, m0+mu).
        cofs: m-offset of this update range inside the coefficient tiles."""
        c16, sr16, s2c = coeffs
        UD = [[SH, 2], [SI, 8], [1, mu]]
        UDsw = [[-SH, 2], [SI, 8], [1, mu]]
        colp = av(p * SJ + m0, list(UD))
        colq = av(q * SJ + m0, list(UD))
        colp_sw = av(SH + p * SJ + m0, list(UDsw))
        colq_sw = av(SH + q * SJ + m0, list(UDsw))
        cb = bass.AP(tensor=c16.tensor, offset=c16.offset + k * mp + cofs,
                     ap=[list(c16.ap[0]), [0, 2], [0, 8], [1, mu]])
        srb = bass.AP(tensor=sr16.tensor, offset=sr16.offset + k * mp + cofs,
                      ap=[list(sr16.ap[0]), [0, 2], [0, 8], [1, mu]])
        s2b = bass.AP(tensor=s2c.tensor, offset=s2c.offset + k * mp + cofs,
                      ap=[list(s2c.ap[0]), [4 * mp, 2], [0, 8], [1, mu]])

        def CW(tag):
            return cp.tile([128, 2, 8, mu], f16, tag=f"{tag}{mu}g{m0}",
                           name=tag)[:]

        tP, uP, tQ, uQ = CW("tP"), CW("uP"), CW("tQ"), CW("uQ")
        TT(tP, srb, colq, ALU.mult)
        TT(uP, s2b, colq_sw, ALU.mult)
        GT(tQ, srb, colp, ALU.mult)
        TT(uQ, s2b, colp_sw, ALU.mult)
        TT(colp, cb, colp, ALU.mult)
        TT(colp, colp, tP, ALU.add)
        TT(colp, colp, uP, ALU.add)
        TT(colq, cb, colq, ALU.mult)
        TT(colq, colq, tQ, ALU.subtract)
        TT(colq, colq, uQ, ALU.add)

        # Hermitian row restore (merged rows p,q): rows <- conj(cols).
        # The (p,q)/(q,p) entries race within the merged ops but are
        # explicitly zeroed below.
        dROW = [[(q - p) * SI, 2], [SJ, 8], [1, mu]]
        sCOL = [[(q - p) * SJ, 2], [SI, 8], [1, mu]]
        nc.scalar.copy(av(p * SI + m0, list(dROW)), av(p * SJ + m0, list(sCOL)))
        n1v = bass.AP(tensor=n1ap.tensor, offset=n1ap.offset,
                      ap=[list(n1ap.ap[0]), [0, 2], [0, 8], [1, mu]])
        TT(av(SH + p * SI + m0, list(dROW)), av(SH + p * SJ + m0, list(sCOL)),
           n1v, ALU.mult)

        # diag mirror (fp16 <- f32 Dg) + annihilated-entry zeros
        mdst = av(p * (SI + SJ) + m0, [[(q - p) * (SI + SJ), 2], [1, mu]])
        msrc = dv(p * NM + m0, [[(q - p) * NM, 2], [1, mu]])
        nc.gpsimd.tensor_copy(mdst, msrc)
        nc.scalar.memzero(av(SH + p * (SI + SJ) + m0,
                             [[(q - p) * SI, 2], [(q - p) * SJ, 2], [1, mu]]))
        nc.gpsimd.memset(av(p * SI + q * SJ + m0,
                            [[(q - p) * (SI - SJ), 2], [1, mu]]), 0.0)

    # ---- sweeps ----
    # F-part: two m-groups, params of each group's next round software-
    # pipelined into the other group's pair updates (fills the params-chain
    # latency with independent column-update work).
    GRP = ((0, 48), (48, 48))

    def pair_emitters(r, m0, mu, mp, co, cofs=0):
        return [(lambda k=k, p=p, q=q: emit_pair(k, p, q, m0, mu, mp, co, cofs))
                for k, (p, q) in enumerate(_xor_pairs(r))]

    def zip_emit(blocks, thunks):
        per = (len(thunks) + len(blocks) - 1) // len(blocks) if thunks else 0
        for i, b in enumerate(blocks):
            b()
            for t in thunks[i * per:(i + 1) * per]:
                t()

    rounds = [r for s in range(N_FULL) for r in range(1, 8)]
    co0, ops0 = make_params(rounds[0], 0, 48)
    for t in ops0:
        t()
    co1, pending1 = make_params(rounds[0], 48, 48)
    for idx, r in enumerate(rounds):
        last_pt = idx == len(rounds) - 1
        nxt = rounds[idx + 1] if idx + 1 < len(rounds) else None
        # G0 updates for round r; zip in G1's params for round r
        mu0 = MRHO if last_pt else 48
        zip_emit(pair_emitters(r, 0, mu0, 48, co0), pending1)
        # params for the next G0 round (or the first rho-only round)
        if nxt is not None:
            co0n, ops0n = make_params(nxt, 0, 48)
        else:
            co0n, ops0n = make_params(1, 0, MRHO)   # R-part round 1
        if last_pt:
            for t in ops0n:
                t()
        else:
            zip_emit(pair_emitters(r, 48, 48, 48, co1), ops0n)
            co1, pending1 = make_params(nxt, 48, 48)
        co0 = co0n

    # R-part: rho-only sweeps, two 16-matrix subgroups ping-ponged like the
    # F-part so each subgroup's params hide under the other's updates.
    rrounds = [r for s in range(N_RHO) for r in range(1, 8)]
    # co0 currently holds params emitted for R round 1 over the full rho
    # range [0, 32) (produced during the last F round) -> use for both
    # subgroups' first round via slicing-compatible mp=32 views.
    rc0 = rc1 = co0
    rmp = MRHO
    pend1 = []
    for idx, r in enumerate(rrounds):
        nxt = rrounds[idx + 1] if idx + 1 < len(rrounds) else None
        zip_emit(pair_emitters(r, 0, 16, rmp if idx == 0 else 16, rc0), pend1)
        if nxt is not None:
            rc0n, ops0n = make_params(nxt, 0, 16)
        else:
            rc0n, ops0n = None, []
        zip_emit(pair_emitters(r, 16, 16, rmp if idx == 0 else 16, rc1,
                               cofs=16 if idx == 0 else 0), ops0n)
        if nxt is not None:
            rc1n, pend1 = make_params(nxt, 16, 16)
            rc0, rc1 = rc0n, rc1n

    # ---- pt_a / pt_c diag min/max over i (final after F-part) ----
    mn = main.tile([128, 2 * NTILES], f32, name="mn")[:]
    mx = main.tile([128, 2 * NTILES], f32, name="mx")[:]
    ptv = dv(NTILES, [[1, 2 * NTILES], [NM, 8]])
    nc.vector.tensor_reduce(mn, ptv, mybir.AxisListType.X, ALU.min)
    nc.vector.tensor_reduce(mx, ptv, mybir.AxisListType.X, ALU.max)
    mu_min = mn[:, 0:NTILES]
    mu_max = mx[:, 0:NTILES]
    nu_min = mn[:, NTILES:2 * NTILES]
    nu_max = mx[:, NTILES:2 * NTILES]

    # ---- perturbative rho diag correction ----
    M = MRHO

    def Q(tag, dt=f32):
        return main.tile([128, 8, 8, M], dt, tag=tag, name=tag)[:]

    SQ, S, dif, dif2 = Q("pSQ"), Q("pS"), Q("pdif"), Q("pdif2")
    corr = main.tile([128, 8, M], f32, tag="pcorr", name="pcorr")[:]
    # |a_ij|^2 in fp32 from fp16 halves
    nc.scalar.activation(SQ, av(0, [[SI, 8], [SJ, 8], [1, M]]), ACT.Square)
    nc.scalar.activation(S, av(SH, [[SI, 8], [SJ, 8], [1, M]]), ACT.Square)
    TT(S, S, SQ, ALU.add)
    TT(dif, dv(0, [[NM, 8], [0, 8], [1, M]]),
       dv(0, [[0, 8], [NM, 8], [1, M]]), ALU.subtract)         # d_i - d_j
    nc.scalar.activation(dif2, dif, ACT.Square)
    dif2_flat = bass.AP(tensor=dif2.tensor, offset=dif2.offset,
                        ap=[list(dif2.ap[0]), [1, 64 * M]])
    nc.vector.tensor_scalar(dif2_flat, dif2_flat, PDELTA, None, ALU.add)
    nc.vector.reciprocal(dif2, dif2)                           # R
    GT(SQ, S, dif, ALU.mult)                                   # W = S*dif
    TT(SQ, SQ, dif2, ALU.mult)
    # corr_i = sum_j W[i, j, m]: reduce over j (view with j last)
    Wv = bass.AP(tensor=SQ.tensor, offset=SQ.offset,
                 ap=[list(SQ.ap[0]), [8 * M, 8], [1, M], [M, 8]])
    nc.vector.tensor_reduce(corr, Wv, mybir.AxisListType.X, ALU.add)
    TT(dv(0, [[NM, 8], [1, M]]), dv(0, [[NM, 8], [1, M]]), corr, ALU.add)

    # ---- rho diag sort (ascending) ----
    tmin = main.tile([128, MRHO], f32, name="tmin")[:]
    for (i, j) in _CE8:
        di = dv(i * NM, [[1, MRHO]])
        dj = dv(j * NM, [[1, MRHO]])
        TT(tmin, di, dj, ALU.min)
        TT(dj, di, dj, ALU.max)
        nc.gpsimd.tensor_copy(di, tmin)

    # ---- loss assembly ----
    def L(name):
        return main.tile([128, NTILES], f32, tag=name, name=name)[:]

    w_min = dv(0, [[1, MRHO]])
    w_max = dv(7 * NM, [[1, MRHO]])
    b0, b1, acc, t1, t2_, t3 = L("b0"), L("b1"), L("acc"), L("t1"), L("t2"), L("t3")

    nc.vector.tensor_scalar(b0, w_min, -8.0, 1.0, ALU.mult, ALU.add)
    nc.vector.reciprocal(b0, b0)
    nc.vector.tensor_scalar(b1, w_max, -8.0, 1.0, ALU.mult, ALU.add)
    nc.vector.reciprocal(b1, b1)

    assert 1 <= k0 <= 8 and 1 <= k1 <= 8
    nc.gpsimd.tensor_copy(t1, dv(0, [[1, MRHO]]))
    for i in range(1, k0):
        TT(t1, t1, dv(i * NM, [[1, MRHO]]), ALU.add)
    nc.gpsimd.tensor_copy(t2_, dv(7 * NM, [[1, MRHO]]))
    for i in range(6, 7 - k1, -1):
        TT(t2_, t2_, dv(i * NM, [[1, MRHO]]), ALU.add)
    nc.vector.tensor_scalar(t1, t1, -k0 / 8.0, None, ALU.add)
    TT(t1, t1, b0, ALU.mult)
    nc.vector.tensor_scalar(t2_, t2_, -k1 / 8.0, None, ALU.add)
    TT(t2_, t2_, b1, ALU.mult)
    TT(t1, t1, t2_, ALU.add)
    nc.vector.tensor_scalar(t1, t1, (k0 + k1) / 8.0, None, ALU.add)
    TT(acc, t1, t1, ALU.mult)
    for beta, ext in ((b0, mu_min), (b1, mu_max), (b0, nu_min), (b1, nu_max)):
        nc.vector.tensor_scalar(t3, ext, -0.125, None, ALU.add)
        TT(t3, t3, beta, ALU.mult)
        nc.vector.tensor_scalar(t3, t3, 0.125, None, ALU.add)
        TT(t3, t3, t3, ALU.mult)
        TT(acc, acc, t3, ALU.add)

    nc.sync.dma_start(out=out_d[:, :], in_=acc)


_prog_cache = {}


def kernel(rho_vec, rank0, rank1):
    rho_vec = np.asarray(rho_vec, dtype=np.float32)
    k0 = D - int(rank0)
    k1 = D - int(rank1)
    in_maps = _host_prep(rho_vec)

    from concourse.bass_utils import run_bass_kernel_spmd
    key = (k0, k1)
    if key not in _prog_cache:
        _prog_cache[key] = _build_program(k0, k1)
    nc = _prog_cache[key]
    res = run_bass_kernel_spmd(nc, in_maps, core_ids=list(range(NCORES)))
    return np.concatenate(
        [np.asarray(res.results[c]["out"]).T.reshape(-1) for c in range(NCORES)]
    ).astype(np.float32)


# revision 40
# speedup vs baseline: 2.3167x; 1.0033x over previous
"""Trainium2 Bass kernel for nn_BESNumEigen3qubitModel (v2).

Math reduction (exact): every eigvalsh in the reference reduces to
eigenvalues of 3 Hermitian 8x8 matrices per batch element: rho, pt_a(rho),
pt_c(rho) (see kernel_baseline for the derivation).

Device algorithm (per core: 4096 batch elems -> 128 partitions x 32 tiles,
3 matrix types -> 96 matrices per partition):
  - Matrix data fp16, layout [128, h(2), i(8), j(8), m(96)] with the matrix
    index m LAST (stride 1) so every DVE operand is packed 2-byte -> 2x/4x
    DVE throughput. Authoritative diagonal kept in f32 [128, 8, 96].
  - Cyclic complex Jacobi in XOR-pair rounds: per round the 4 pairs' rotation
    params are computed batched (exact: a round's 2x2 blocks are disjoint),
    then per-pair column update + Hermitian row restore sequentially.
  - 2 full sweeps (all 96), 1 extra rho-only sweep, then a 2nd-order
    perturbative diag correction for rho from the residual off-diagonal.
    PT matrices only need extremal eigenvalues (their final-round column
    updates are dead and skipped).
  - Sort rho diag (Batcher network), min/max-reduce PT diags, assemble loss.
"""

import numpy as np

D = 8
BATCH = 32768
NCORES = 8
PER_CORE = BATCH // NCORES       # 4096
NTILES = PER_CORE // 128         # 32 tiles per core
NM = 3 * NTILES                  # 96 matrices per partition (type-major)
MR = NM                          # full-round matrix count
MRHO = NTILES                    # rho-only count

# elem strides inside the fp16 matrix tile [2(h), 8(i), 8(j), NM(m)]
SM, SJ, SI, SH = 1, NM, 8 * NM, 64 * NM
ASIZE = 2 * 8 * 8 * NM           # 12288
PDELTA = 1e-6                    # perturbative-correction regularizer

_f32 = np.float32


# ---------------------------------------------------------------- host prep --

def _gellmann_basis(d):
    mats = []
    for j in range(d):
        for k in range(j + 1, d):
            m = np.zeros((d, d), np.complex128); m[j, k] = 1; m[k, j] = 1
            mats.append(m)
    for j in range(d):
        for k in range(j + 1, d):
            m = np.zeros((d, d), np.complex128); m[j, k] = -1j; m[k, j] = 1j
            mats.append(m)
    for l in range(1, d):
        m = np.zeros((d, d), np.complex128)
        m[np.arange(l), np.arange(l)] = 1
        m[l, l] = -l
        mats.append(np.sqrt(2.0 / (l * (l + 1))) * m)
    return np.stack(mats)


def _entry_perm(kind):
    p = np.zeros(64, np.int64)
    for i in range(8):
        for j in range(8):
            if kind == 'a':
                i2, j2 = (j & 4) | (i & 3), (i & 4) | (j & 3)
            else:
                i2, j2 = (i & 6) | (j & 1), (j & 6) | (i & 1)
            p[i * 8 + j] = i2 * 8 + j2
    return p


def _build_maps():
    """[64, 384] f32: (vec,1) -> 128 floats (f = h*64 + i*8 + j) of each of
    rho, pt_a(rho), pt_c(rho)."""
    G = _gellmann_basis(D)
    B = np.zeros((64, 128), np.float64)
    for k in range(63):
        B[k, :64] = G[k].real.reshape(-1)
        B[k, 64:] = G[k].imag.reshape(-1)
    B[63, :64] = (np.eye(D) / D).reshape(-1)

    def float_perm(kind):
        e = _entry_perm(kind)
        return np.concatenate([e, 64 + e])

    M3 = np.concatenate([B, B[:, float_perm('a')], B[:, float_perm('c')]], axis=1)
    return M3.astype(_f32)


_M3 = None


def _host_prep(rho_vec):
    global _M3
    if _M3 is None:
        _M3 = _build_maps()
    vec = rho_vec.astype(np.float64)
    vec = vec / np.linalg.norm(vec, axis=-1, keepdims=True)
    vec_aug = np.concatenate(
        [vec.astype(_f32), np.ones((vec.shape[0], 1), _f32)], axis=1)
    flat = vec_aug @ _M3                                   # [B, 384] f32
    arr = flat.reshape(NCORES, NTILES, 128, 3, 128)        # [core,tile,part,type,f]
    ins = []
    diag_f = np.array([i * 8 + i for i in range(8)])
    for c in range(NCORES):
        a = arr[c]
        # fp16 matrices: [part, f, type, tile] -> [128, f*96 + type*32 + tile]
        m16 = np.ascontiguousarray(
            a.transpose(1, 3, 2, 0).reshape(128, 128 * NM)).astype(np.float16)
        # f32 diag: [part, i, type, tile] -> [128, i*96 + m]
        dg = np.ascontiguousarray(
            a[:, :, :, diag_f].transpose(1, 3, 2, 0).reshape(128, 8 * NM)
        ).astype(_f32)
        ins.append({"mats": m16, "diag": dg})
    return ins


# ------------------------------------------------------------ device kernel --

def _xor_pairs(r):
    return [(i, i ^ r) for i in range(8) if i < (i ^ r)]


def _enum_bits(r):
    """Enumeration bit-steps (descending) for pset = {p: bit_bmax(r)(p)=0},
    enumerated in ascending-p order."""
    bmax = 4 if r >= 4 else (2 if r >= 2 else 1)
    return [b for b in (4, 2, 1) if b != bmax]


# Batcher odd-even mergesort network for 8 elements (19 comparators)
_CE8 = [(0, 1), (2, 3), (4, 5), (6, 7), (0, 2), (1, 3), (4, 6), (5, 7),
        (1, 2), (5, 6), (0, 4), (1, 5), (2, 6), (3, 7), (2, 4), (3, 5),
        (1, 2), (3, 4), (5, 6)]

N_FULL = 2      # full sweeps (all 3 matrix types)
N_RHO = 1       # extra rho-only sweeps


def _build_program(k0, k1):
    import concourse.bass as bass
    import concourse.bacc as bacc
    import concourse.mybir as mybir
    from concourse.tile import TileContext
    from contextlib import ExitStack

    f32 = mybir.dt.float32
    f16 = mybir.dt.float16
    ALU = mybir.AluOpType
    ACT = mybir.ActivationFunctionType

    nc = bacc.Bacc("TRN2")
    mats_d = nc.dram_tensor("mats", [128, ASIZE], f16, kind="ExternalInput")
    diag_d = nc.dram_tensor("diag", [128, 8 * NM], f32, kind="ExternalInput")
    out_d = nc.dram_tensor("out", [128, NTILES], f32, kind="ExternalOutput")

    with ExitStack() as ctx:
        tc = ctx.enter_context(TileContext(nc))
        main = ctx.enter_context(tc.tile_pool(name="main", bufs=1))
        pp = ctx.enter_context(tc.tile_pool(name="pp", bufs=2))
        cp = ctx.enter_context(tc.tile_pool(name="cp", bufs=2))

        A = main.tile([128, ASIZE], f16, name="A")
        Dg = main.tile([128, 8 * NM], f32, name="Dg")
        Aap = A[:]
        Dap = Dg[:]
        pdim = list(Aap.ap[0])

        def av(offset, dims):
            return bass.AP(tensor=Aap.tensor, offset=Aap.offset + offset,
                           ap=[pdim] + dims)

        def dv(offset, dims):
            return bass.AP(tensor=Dap.tensor, offset=Dap.offset + offset,
                           ap=[list(Dap.ap[0])] + dims)

        nc.sync.dma_start(out=Dg[:], in_=diag_d[:, :])
        NCHUNK = 4
        for ch in range(NCHUNK):
            w = ASIZE // NCHUNK
            nc.sync.dma_start(out=av(ch * w, [[1, w]]),
                              in_=mats_d[:, ch * w:(ch + 1) * w])

        eps30 = main.tile([128, 1], f32, name="eps30")
        nc.vector.memset(eps30[:], 1e-30)
        eps35 = main.tile([128, 1], f32, name="eps35")
        nc.vector.memset(eps35[:], 1e-35)

        with nc.allow_low_precision(reason="fp16 Jacobi data by design"):
            _emit_jacobi(nc, bass, mybir, main, pp, cp, av, dv,
                         eps30, eps35, out_d, k0, k1)

    nc.finalize()
    return nc


def _emit_jacobi(nc, bass, mybir, main, pp, cp, av, dv, eps30, eps35,
                 out_d, k0, k1):
    f32 = mybir.dt.float32
    f16 = mybir.dt.float16
    ALU = mybir.AluOpType
    ACT = mybir.ActivationFunctionType
    TT = nc.vector.tensor_tensor
    GT = nc.gpsimd.tensor_tensor
    STT = nc.vector.scalar_tensor_tensor

    neg1 = main.tile([128, NM], f16, name="neg1")
    nc.vector.memset(neg1[:], -1.0)
    n1ap = neg1[:]

    def make_params(r, m0, mp):
        """Rotation params for round r, matrices m in [m0, m0+mp).
        Returns ((c16, sr16, s2c), [op thunks]) - thunks emit one op each,
        in dependency order, so callers can interleave them with other work."""
        b1, b2 = _enum_bits(r)  # descending

        def merged(dims):
            if dims[0][0] == 2 * dims[1][0]:
                return [[dims[1][0], 4]] + dims[2:]
            return dims

        sgn = lambda b: -1 if (r & b) else 1
        xdims = merged([[b1 * SI + sgn(b1) * b1 * SJ, 2],
                        [b2 * SI + sgn(b2) * b2 * SJ, 2], [1, mp]])
        Xv = av(r * SJ + m0, list(xdims))
        Yv = av(SH + r * SJ + m0, list(xdims))
        appv = dv(m0, merged([[b1 * NM, 2], [b2 * NM, 2], [1, mp]]))
        aqqv = dv(r * NM + m0, merged([[sgn(b1) * b1 * NM, 2],
                                       [sgn(b2) * b2 * NM, 2], [1, mp]]))

        def P(tag, dt=f32):
            return pp.tile([128, 4, mp], dt, tag=f"{tag}{mp}g{m0}", name=tag)[:]

        sqx, sqy, n2p, g = P("sqx"), P("sqy"), P("n2p"), P("g")
        gsq, s2, h, ag = P("gsq"), P("s2"), P("h"), P("ag")
        den, T, sg, T2 = P("den"), P("T"), P("sg"), P("T2")
        t2, cden, u, urb2 = P("t2"), P("cden"), P("u"), P("urb2")
        tb = P("tb")
        c16 = P("c16", f16)
        sr16 = P("sr16", f16)
        s2c = pp.tile([128, 2, 4, mp], f16, tag=f"s2c{mp}g{m0}", name="s2c")[:]

        ops = [
            lambda: nc.scalar.activation(sqx, Xv, ACT.Square, scale=2.0),
            lambda: nc.scalar.activation(sqy, Yv, ACT.Square, scale=2.0),
            lambda: TT(g, appv, aqqv, ALU.subtract),
            lambda: nc.scalar.activation(ag, g, ACT.Abs),
            lambda: nc.scalar.sign(sg, g, bias=eps35[:]),
            lambda: nc.scalar.activation(gsq, g, ACT.Square),
            lambda: TT(n2p, sqx, sqy, ALU.add),
            lambda: TT(s2, gsq, n2p, ALU.add),
            lambda: nc.scalar.activation(h, s2, ACT.Sqrt, bias=eps30[:]),
            lambda: GT(den, ag, h, ALU.add),
            lambda: nc.vector.reciprocal(T, den),
            lambda: GT(T2, T, T, ALU.mult),
            lambda: TT(t2, n2p, T2, ALU.mult),
            lambda: nc.scalar.activation(cden, t2, ACT.Sqrt, bias=1.0),
            lambda: nc.vector.reciprocal(c16, cden),
            lambda: GT(u, T, sg, ALU.mult),
            lambda: STT(urb2, u, 2.0, c16, ALU.mult, ALU.mult),
            lambda: TT(sr16, urb2, Xv, ALU.mult),
            lambda: TT(s2c[:, 0], urb2, Yv, ALU.mult),
            lambda: nc.vector.tensor_scalar(s2c[:, 1], s2c[:, 0], -1.0,
                                            None, ALU.mult),
            lambda: STT(tb, u, 0.5, n2p, ALU.mult, ALU.mult),
            lambda: GT(appv, appv, tb, ALU.add),
            lambda: GT(aqqv, aqqv, tb, ALU.subtract),
        ]
        return (c16, sr16, s2c), ops

    def emit_pair(k, p, q, m0, mu, mp, coeffs, cofs=0):
        """Column update + Hermitian restore for pair (p, q), m in [m0, m0+mu).
        cofs: m-offset of this update range inside the coefficient tiles."""
        c16, sr16, s2c = coeffs
        UD = [[SH, 2], [SI, 8], [1, mu]]
        UDsw = [[-SH, 2], [SI, 8], [1, mu]]
        colp = av(p * SJ + m0, list(UD))
        colq = av(q * SJ + m0, list(UD))
        colp_sw = av(SH + p * SJ + m0, list(UDsw))
        colq_sw = av(SH + q * SJ + m0, list(UDsw))
        cb = bass.AP(tensor=c16.tensor, offset=c16.offset + k * mp + cofs,
                     ap=[list(c16.ap[0]), [0, 2], [0, 8], [1, mu]])
        srb = bass.AP(tensor=sr16.tensor, offset=sr16.offset + k * mp + cofs,
                      ap=[list(sr16.ap[0]), [0, 2], [0, 8], [1, mu]])
        s2b = bass.AP(tensor=s2c.tensor, offset=s2c.offset + k * mp + cofs,
                      ap=[list(s2c.ap[0]), [4 * mp, 2], [0, 8], [1, mu]])

        def CW(tag):
            return cp.tile([128, 2, 8, mu], f16, tag=f"{tag}{mu}g{m0}",
                           name=tag)[:]

        tP, uP, tQ, uQ = CW("tP"), CW("uP"), CW("tQ"), CW("uQ")
        TT(tP, srb, colq, ALU.mult)
        TT(uP, s2b, colq_sw, ALU.mult)
        GT(tQ, srb, colp, ALU.mult)
        TT(uQ, s2b, colp_sw, ALU.mult)
        TT(colp, cb, colp, ALU.mult)
        TT(colp, colp, tP, ALU.add)
        TT(colp, colp, uP, ALU.add)
        TT(colq, cb, colq, ALU.mult)
        TT(colq, colq, tQ, ALU.subtract)
        TT(colq, colq, uQ, ALU.add)

        # Hermitian row restore (merged rows p,q): rows <- conj(cols).
        # The (p,q)/(q,p) entries race within the merged ops but are
        # explicitly zeroed below.
        dROW = [[(q - p) * SI, 2], [SJ, 8], [1, mu]]
        sCOL = [[(q - p) * SJ, 2], [SI, 8], [1, mu]]
        nc.scalar.copy(av(p * SI + m0, list(dROW)), av(p * SJ + m0, list(sCOL)))
        n1v = bass.AP(tensor=n1ap.tensor, offset=n1ap.offset,
                      ap=[list(n1ap.ap[0]), [0, 2], [0, 8], [1, mu]])
        TT(av(SH + p * SI + m0, list(dROW)), av(SH + p * SJ + m0, list(sCOL)),
           n1v, ALU.mult)

        # diag mirror (fp16 <- f32 Dg) + annihilated-entry zeros
        mdst = av(p * (SI + SJ) + m0, [[(q - p) * (SI + SJ), 2], [1, mu]])
        msrc = dv(p * NM + m0, [[(q - p) * NM, 2], [1, mu]])
        nc.gpsimd.tensor_copy(mdst, msrc)
        nc.scalar.memzero(av(SH + p * (SI + SJ) + m0,
                             [[(q - p) * SI, 2], [(q - p) * SJ, 2], [1, mu]]))
        nc.gpsimd.memset(av(p * SI + q * SJ + m0,
                            [[(q - p) * (SI - SJ), 2], [1, mu]]), 0.0)

    # ---- sweeps ----
    # F-part: two m-groups, params of each group's next round software-
    # pipelined into the other group's pair updates (fills the params-chain
    # latency with independent column-update work).
    GRP = ((0, 48), (48, 48))

    def pair_emitters(r, m0, mu, mp, co, cofs=0):
        return [(lambda k=k, p=p, q=q: emit_pair(k, p, q, m0, mu, mp, co, cofs))
                for k, (p, q) in enumerate(_xor_pairs(r))]

    def zip_emit(blocks, thunks):
        per = (len(thunks) + len(blocks) - 1) // len(blocks) if thunks else 0
        for i, b in enumerate(blocks):
            b()
            for t in thunks[i * per:(i + 1) * per]:
                t()

    rounds = [r for s in range(N_FULL) for r in range(1, 8)]
    co0, ops0 = make_params(rounds[0], 0, 48)
    for t in ops0:
        t()
    co1, pending1 = make_params(rounds[0], 48, 48)
    for idx, r in enumerate(rounds):
        last_pt = idx == len(rounds) - 1
        nxt = rounds[idx + 1] if idx + 1 < len(rounds) else None
        # G0 updates for round r; zip in G1's params for round r
        mu0 = MRHO if last_pt else 48
        zip_emit(pair_emitters(r, 0, mu0, 48, co0), pending1)
        # params for the next G0 round (or the first rho-only round)
        if nxt is not None:
            co0n, ops0n = make_params(nxt, 0, 48)
        else:
            co0n, ops0n = make_params(1, 0, MRHO)   # R-part round 1
        if last_pt:
            for t in ops0n:
                t()
        else:
            zip_emit(pair_emitters(r, 48, 48, 48, co1), ops0n)
            co1, pending1 = make_params(nxt, 48, 48)
        co0 = co0n

    # R-part: rho-only sweeps, two 16-matrix subgroups ping-ponged like the
    # F-part so each subgroup's params hide under the other's updates.
    rrounds = [r for s in range(N_RHO) for r in range(1, 8)]
    # co0 currently holds params emitted for R round 1 over the full rho
    # range [0, 32) (produced during the last F round) -> use for both
    # subgroups' first round via slicing-compatible mp=32 views.
    rc0 = rc1 = co0
    rmp = MRHO
    pend1 = []
    for idx, r in enumerate(rrounds):
        nxt = rrounds[idx + 1] if idx + 1 < len(rrounds) else None
        zip_emit(pair_emitters(r, 0, 16, rmp if idx == 0 else 16, rc0), pend1)
        if nxt is not None:
            rc0n, ops0n = make_params(nxt, 0, 16)
        else:
            rc0n, ops0n = None, []
        zip_emit(pair_emitters(r, 16, 16, rmp if idx == 0 else 16, rc1,
                               cofs=16 if idx == 0 else 0), ops0n)
        if nxt is not None:
            rc1n, pend1 = make_params(nxt, 16, 16)
            rc0, rc1 = rc0n, rc1n

    # ---- pt_a / pt_c diag min/max over i (final after F-part) ----
    mn = main.tile([128, 2 * NTILES], f32, name="mn")[:]
    mx = main.tile([128, 2 * NTILES], f32, name="mx")[:]
    ptv = dv(NTILES, [[1, 2 * NTILES], [NM, 8]])
    nc.vector.tensor_reduce(mn, ptv, mybir.AxisListType.X, ALU.min)
    nc.vector.tensor_reduce(mx, ptv, mybir.AxisListType.X, ALU.max)
    mu_min = mn[:, 0:NTILES]
    mu_max = mx[:, 0:NTILES]
    nu_min = mn[:, NTILES:2 * NTILES]
    nu_max = mx[:, NTILES:2 * NTILES]

    # ---- perturbative rho diag correction ----
    M = MRHO

    def Q(tag, dt=f32):
        return main.tile([128, 8, 8, M], dt, tag=tag, name=tag)[:]

    SQ, S, dif, dif2 = Q("pSQ"), Q("pS"), Q("pdif"), Q("pdif2")
    corr = main.tile([128, 8, M], f32, tag="pcorr", name="pcorr")[:]
    # |a_ij|^2 in fp32 from fp16 halves
    nc.scalar.activation(SQ, av(0, [[SI, 8], [SJ, 8], [1, M]]), ACT.Square)
    nc.scalar.activation(S, av(SH, [[SI, 8], [SJ, 8], [1, M]]), ACT.Square)
    TT(S, S, SQ, ALU.add)
    TT(dif, dv(0, [[NM, 8], [0, 8], [1, M]]),
       dv(0, [[0, 8], [NM, 8], [1, M]]), ALU.subtract)         # d_i - d_j
    nc.scalar.activation(dif2, dif, ACT.Square)
    dif2_flat = bass.AP(tensor=dif2.tensor, offset=dif2.offset,
                        ap=[list(dif2.ap[0]), [1, 64 * M]])
    nc.vector.tensor_scalar(dif2_flat, dif2_flat, PDELTA, None, ALU.add)
    nc.vector.reciprocal(dif2, dif2)                           # R
    GT(SQ, S, dif, ALU.mult)                                   # W = S*dif
    TT(SQ, SQ, dif2, ALU.mult)
    # corr_i = sum_j W[i, j, m]: reduce over j (view with j last)
    Wv = bass.AP(tensor=SQ.tensor, offset=SQ.offset,
                 ap=[list(SQ.ap[0]), [8 * M, 8], [1, M], [M, 8]])
    nc.vector.tensor_reduce(corr, Wv, mybir.AxisListType.X, ALU.add)
    TT(dv(0, [[NM, 8], [1, M]]), dv(0, [[NM, 8], [1, M]]), corr, ALU.add)

    # ---- rho diag sort (ascending), in-place 3-op comparators ----
    loc = {i: dv(i * NM, [[1, MRHO]]) for i in range(8)}
    tmin = main.tile([128, MRHO], f32, name="tmin")[:]
    for (i, j) in _CE8:
        di, dj = loc[i], loc[j]
        TT(tmin, di, dj, ALU.min)
        TT(dj, di, dj, ALU.max)
        nc.gpsimd.tensor_copy(di, tmin)

    # ---- loss assembly (fused STT forms, tree accumulation) ----
    def L(name):
        return main.tile([128, NTILES], f32, tag=name, name=name)[:]

    STTG = nc.gpsimd.scalar_tensor_tensor
    b0, b1 = L("b0"), L("b1")
    nc.vector.tensor_scalar(b0, loc[0], -8.0, 1.0, ALU.mult, ALU.add)
    nc.vector.reciprocal(b0, b0)
    nc.vector.tensor_scalar(b1, loc[7], -8.0, 1.0, ALU.mult, ALU.add)
    nc.vector.reciprocal(b1, b1)

    assert 1 <= k0 <= 8 and 1 <= k1 <= 8
    t1, t2_, acc = L("t1"), L("t2"), L("acc")
    sA, sB = L("sA"), L("sB")
    if k0 == 4:
        TT(sA, loc[0], loc[1], ALU.add)
        GT(sB, loc[2], loc[3], ALU.add)
        TT(t1, sA, sB, ALU.add)
    else:
        nc.gpsimd.tensor_copy(t1, loc[0])
        for i in range(1, k0):
            TT(t1, t1, loc[i], ALU.add)
    if k1 == 4:
        TT(sA, loc[7], loc[6], ALU.add)
        GT(sB, loc[5], loc[4], ALU.add)
        TT(t2_, sA, sB, ALU.add)
    else:
        nc.gpsimd.tensor_copy(t2_, loc[7])
        for i in range(6, 7 - k1, -1):
            TT(t2_, t2_, loc[i], ALU.add)
    u0, u1 = L("u0"), L("u1")
    STT(u0, t1, -k0 / 8.0, b0, ALU.add, ALU.mult)     # b0*(S-k0/8)
    STT(u1, t2_, -k1 / 8.0, b1, ALU.add, ALU.mult)
    TT(u0, u0, u1, ALU.add)
    nc.vector.tensor_scalar(u0, u0, (k0 + k1) / 8.0, None, ALU.add)  # l01
    TT(acc, u0, u0, ALU.mult)
    t3s = [L(f"t3{i}") for i in range(4)]
    for n, (beta, ext) in enumerate(
            ((b0, mu_min), (b1, mu_max), (b0, nu_min), (b1, nu_max))):
        t3 = t3s[n]
        STT(t3, ext, -0.125, beta, ALU.add, ALU.mult)
        nc.vector.tensor_scalar(t3, t3, 0.125, None, ALU.add)
        if n % 2 == 0:
            TT(t3, t3, t3, ALU.mult)
        else:
            GT(t3, t3, t3, ALU.mult)
    TT(t3s[0], t3s[0], t3s[1], ALU.add)
    GT(t3s[2], t3s[2], t3s[3], ALU.add)
    TT(t3s[0], t3s[0], t3s[2], ALU.add)
    TT(acc, acc, t3s[0], ALU.add)

    nc.sync.dma_start(out=out_d[:, :], in_=acc)


_prog_cache = {}


def kernel(rho_vec, rank0, rank1):
    rho_vec = np.asarray(rho_vec, dtype=np.float32)
    k0 = D - int(rank0)
    k1 = D - int(rank1)
    in_maps = _host_prep(rho_vec)

    from concourse.bass_utils import run_bass_kernel_spmd
    key = (k0, k1)
    if key not in _prog_cache:
        _prog_cache[key] = _build_program(k0, k1)
    nc = _prog_cache[key]
    res = run_bass_kernel_spmd(nc, in_maps, core_ids=list(range(NCORES)))
    return np.concatenate(
        [np.asarray(res.results[c]["out"]).T.reshape(-1) for c in range(NCORES)]
    ).astype(np.float32)


# revision 41
# speedup vs baseline: 2.3253x; 1.0037x over previous
"""Trainium2 Bass kernel for nn_BESNumEigen3qubitModel (v2).

Math reduction (exact): every eigvalsh in the reference reduces to
eigenvalues of 3 Hermitian 8x8 matrices per batch element: rho, pt_a(rho),
pt_c(rho) (see kernel_baseline for the derivation).

Device algorithm (per core: 4096 batch elems -> 128 partitions x 32 tiles,
3 matrix types -> 96 matrices per partition):
  - Matrix data fp16, layout [128, h(2), i(8), j(8), m(96)] with the matrix
    index m LAST (stride 1) so every DVE operand is packed 2-byte -> 2x/4x
    DVE throughput. Authoritative diagonal kept in f32 [128, 8, 96].
  - Cyclic complex Jacobi in XOR-pair rounds: per round the 4 pairs' rotation
    params are computed batched (exact: a round's 2x2 blocks are disjoint),
    then per-pair column update + Hermitian row restore sequentially.
  - 2 full sweeps (all 96), 1 extra rho-only sweep, then a 2nd-order
    perturbative diag correction for rho from the residual off-diagonal.
    PT matrices only need extremal eigenvalues (their final-round column
    updates are dead and skipped).
  - Sort rho diag (Batcher network), min/max-reduce PT diags, assemble loss.
"""

import numpy as np

D = 8
BATCH = 32768
NCORES = 8
PER_CORE = BATCH // NCORES       # 4096
NTILES = PER_CORE // 128         # 32 tiles per core
NM = 3 * NTILES                  # 96 matrices per partition (type-major)
MR = NM                          # full-round matrix count
MRHO = NTILES                    # rho-only count

# elem strides inside the fp16 matrix tile [2(h), 8(i), 8(j), NM(m)]
SM, SJ, SI, SH = 1, NM, 8 * NM, 64 * NM
ASIZE = 2 * 8 * 8 * NM           # 12288
PDELTA = 1e-6                    # perturbative-correction regularizer

_f32 = np.float32


# ---------------------------------------------------------------- host prep --

def _gellmann_basis(d):
    mats = []
    for j in range(d):
        for k in range(j + 1, d):
            m = np.zeros((d, d), np.complex128); m[j, k] = 1; m[k, j] = 1
            mats.append(m)
    for j in range(d):
        for k in range(j + 1, d):
            m = np.zeros((d, d), np.complex128); m[j, k] = -1j; m[k, j] = 1j
            mats.append(m)
    for l in range(1, d):
        m = np.zeros((d, d), np.complex128)
        m[np.arange(l), np.arange(l)] = 1
        m[l, l] = -l
        mats.append(np.sqrt(2.0 / (l * (l + 1))) * m)
    return np.stack(mats)


def _entry_perm(kind):
    p = np.zeros(64, np.int64)
    for i in range(8):
        for j in range(8):
            if kind == 'a':
                i2, j2 = (j & 4) | (i & 3), (i & 4) | (j & 3)
            else:
                i2, j2 = (i & 6) | (j & 1), (j & 6) | (i & 1)
            p[i * 8 + j] = i2 * 8 + j2
    return p


def _build_maps():
    """[64, 384] f32: (vec,1) -> 128 floats (f = h*64 + i*8 + j) of each of
    rho, pt_a(rho), pt_c(rho)."""
    G = _gellmann_basis(D)
    B = np.zeros((64, 128), np.float64)
    for k in range(63):
        B[k, :64] = G[k].real.reshape(-1)
        B[k, 64:] = G[k].imag.reshape(-1)
    B[63, :64] = (np.eye(D) / D).reshape(-1)

    def float_perm(kind):
        e = _entry_perm(kind)
        return np.concatenate([e, 64 + e])

    M3 = np.concatenate([B, B[:, float_perm('a')], B[:, float_perm('c')]], axis=1)
    return M3.astype(_f32)


_M3 = None


def _host_prep(rho_vec):
    global _M3
    if _M3 is None:
        _M3 = _build_maps()
    vec = rho_vec.astype(np.float64)
    vec = vec / np.linalg.norm(vec, axis=-1, keepdims=True)
    vec_aug = np.concatenate(
        [vec.astype(_f32), np.ones((vec.shape[0], 1), _f32)], axis=1)
    flat = vec_aug @ _M3                                   # [B, 384] f32
    arr = flat.reshape(NCORES, NTILES, 128, 3, 128)        # [core,tile,part,type,f]
    ins = []
    diag_f = np.array([i * 8 + i for i in range(8)])
    for c in range(NCORES):
        a = arr[c]
        # fp16 matrices: [part, f, type, tile] -> [128, f*96 + type*32 + tile]
        m16 = np.ascontiguousarray(
            a.transpose(1, 3, 2, 0).reshape(128, 128 * NM)).astype(np.float16)
        # f32 diag: [part, i, type, tile] -> [128, i*96 + m]
        dg = np.ascontiguousarray(
            a[:, :, :, diag_f].transpose(1, 3, 2, 0).reshape(128, 8 * NM)
        ).astype(_f32)
        ins.append({"mats": m16, "diag": dg})
    return ins


# ------------------------------------------------------------ device kernel --

def _xor_pairs(r):
    return [(i, i ^ r) for i in range(8) if i < (i ^ r)]


def _enum_bits(r):
    """Enumeration bit-steps (descending) for pset = {p: bit_bmax(r)(p)=0},
    enumerated in ascending-p order."""
    bmax = 4 if r >= 4 else (2 if r >= 2 else 1)
    return [b for b in (4, 2, 1) if b != bmax]


# Batcher odd-even mergesort network for 8 elements (19 comparators)
_CE8 = [(0, 1), (2, 3), (4, 5), (6, 7), (0, 2), (1, 3), (4, 6), (5, 7),
        (1, 2), (5, 6), (0, 4), (1, 5), (2, 6), (3, 7), (2, 4), (3, 5),
        (1, 2), (3, 4), (5, 6)]

N_FULL = 2      # full sweeps (all 3 matrix types)
N_RHO = 1       # extra rho-only sweeps


def _build_program(k0, k1):
    import concourse.bass as bass
    import concourse.bacc as bacc
    import concourse.mybir as mybir
    from concourse.tile import TileContext
    from contextlib import ExitStack

    f32 = mybir.dt.float32
    f16 = mybir.dt.float16
    ALU = mybir.AluOpType
    ACT = mybir.ActivationFunctionType

    nc = bacc.Bacc("TRN2")
    mats_d = nc.dram_tensor("mats", [128, ASIZE], f16, kind="ExternalInput")
    diag_d = nc.dram_tensor("diag", [128, 8 * NM], f32, kind="ExternalInput")
    out_d = nc.dram_tensor("out", [128, NTILES], f32, kind="ExternalOutput")

    with ExitStack() as ctx:
        tc = ctx.enter_context(TileContext(nc))
        main = ctx.enter_context(tc.tile_pool(name="main", bufs=1))
        pp = ctx.enter_context(tc.tile_pool(name="pp", bufs=2))
        cp = ctx.enter_context(tc.tile_pool(name="cp", bufs=2))

        A = main.tile([128, ASIZE], f16, name="A")
        Dg = main.tile([128, 8 * NM], f32, name="Dg")
        Aap = A[:]
        Dap = Dg[:]
        pdim = list(Aap.ap[0])

        def av(offset, dims):
            return bass.AP(tensor=Aap.tensor, offset=Aap.offset + offset,
                           ap=[pdim] + dims)

        def dv(offset, dims):
            return bass.AP(tensor=Dap.tensor, offset=Dap.offset + offset,
                           ap=[list(Dap.ap[0])] + dims)

        nc.sync.dma_start(out=Dg[:], in_=diag_d[:, :])
        NCHUNK = 4
        for ch in range(NCHUNK):
            w = ASIZE // NCHUNK
            nc.sync.dma_start(out=av(ch * w, [[1, w]]),
                              in_=mats_d[:, ch * w:(ch + 1) * w])

        eps30 = main.tile([128, 1], f32, name="eps30")
        nc.vector.memset(eps30[:], 1e-30)
        eps35 = main.tile([128, 1], f32, name="eps35")
        nc.vector.memset(eps35[:], 1e-35)

        with nc.allow_low_precision(reason="fp16 Jacobi data by design"):
            _emit_jacobi(nc, bass, mybir, main, pp, cp, av, dv,
                         eps30, eps35, out_d, k0, k1)

    nc.finalize()
    return nc


def _emit_jacobi(nc, bass, mybir, main, pp, cp, av, dv, eps30, eps35,
                 out_d, k0, k1):
    f32 = mybir.dt.float32
    f16 = mybir.dt.float16
    ALU = mybir.AluOpType
    ACT = mybir.ActivationFunctionType
    TT = nc.vector.tensor_tensor
    GT = nc.gpsimd.tensor_tensor
    STT = nc.vector.scalar_tensor_tensor

    def make_params(r, m0, mp):
        """Rotation params for round r, matrices m in [m0, m0+mp).
        Returns ((c16, sr16, s2c), [op thunks]) - thunks emit one op each,
        in dependency order, so callers can interleave them with other work."""
        b1, b2 = _enum_bits(r)  # descending

        def merged(dims):
            if dims[0][0] == 2 * dims[1][0]:
                return [[dims[1][0], 4]] + dims[2:]
            return dims

        sgn = lambda b: -1 if (r & b) else 1
        xdims = merged([[b1 * SI + sgn(b1) * b1 * SJ, 2],
                        [b2 * SI + sgn(b2) * b2 * SJ, 2], [1, mp]])
        Xv = av(r * SJ + m0, list(xdims))
        Yv = av(SH + r * SJ + m0, list(xdims))
        appv = dv(m0, merged([[b1 * NM, 2], [b2 * NM, 2], [1, mp]]))
        aqqv = dv(r * NM + m0, merged([[sgn(b1) * b1 * NM, 2],
                                       [sgn(b2) * b2 * NM, 2], [1, mp]]))

        def P(tag, dt=f32):
            return pp.tile([128, 4, mp], dt, tag=f"{tag}{mp}g{m0}", name=tag)[:]

        sqx, sqy, n2p, g = P("sqx"), P("sqy"), P("n2p"), P("g")
        gsq, s2, h, ag = P("gsq"), P("s2"), P("h"), P("ag")
        den, T, sg, T2 = P("den"), P("T"), P("sg"), P("T2")
        t2, cden, u, urb2 = P("t2"), P("cden"), P("u"), P("urb2")
        tb = P("tb")
        c16 = P("c16", f16)
        sr16 = P("sr16", f16)
        s2c = pp.tile([128, 2, 4, mp], f16, tag=f"s2c{mp}g{m0}", name="s2c")[:]

        ops = [
            lambda: nc.scalar.activation(sqx, Xv, ACT.Square, scale=2.0),
            lambda: nc.scalar.activation(sqy, Yv, ACT.Square, scale=2.0),
            lambda: TT(g, appv, aqqv, ALU.subtract),
            lambda: nc.scalar.activation(ag, g, ACT.Abs),
            lambda: nc.scalar.sign(sg, g, bias=eps35[:]),
            lambda: nc.scalar.activation(gsq, g, ACT.Square),
            lambda: TT(n2p, sqx, sqy, ALU.add),
            lambda: TT(s2, gsq, n2p, ALU.add),
            lambda: nc.scalar.activation(h, s2, ACT.Sqrt, bias=eps30[:]),
            lambda: GT(den, ag, h, ALU.add),
            lambda: nc.vector.reciprocal(T, den),
            lambda: GT(T2, T, T, ALU.mult),
            lambda: TT(t2, n2p, T2, ALU.mult),
            lambda: nc.scalar.activation(cden, t2, ACT.Sqrt, bias=1.0),
            lambda: nc.vector.reciprocal(c16, cden),
            lambda: GT(u, T, sg, ALU.mult),
            lambda: STT(urb2, u, 2.0, c16, ALU.mult, ALU.mult),
            lambda: TT(sr16, urb2, Xv, ALU.mult),
            lambda: TT(s2c[:, 0], urb2, Yv, ALU.mult),
            lambda: nc.vector.tensor_scalar(s2c[:, 1], s2c[:, 0], -1.0,
                                            None, ALU.mult),
            lambda: STT(tb, u, 0.5, n2p, ALU.mult, ALU.mult),
            lambda: GT(appv, appv, tb, ALU.add),
            lambda: GT(aqqv, aqqv, tb, ALU.subtract),
        ]
        return (c16, sr16, s2c), ops

    def emit_pair(k, p, q, m0, mu, mp, coeffs, cofs=0):
        """Column update + Hermitian restore for pair (p, q), m in [m0, m0+mu).
        cofs: m-offset of this update range inside the coefficient tiles."""
        c16, sr16, s2c = coeffs
        UD = [[SH, 2], [SI, 8], [1, mu]]
        UDsw = [[-SH, 2], [SI, 8], [1, mu]]
        colp = av(p * SJ + m0, list(UD))
        colq = av(q * SJ + m0, list(UD))
        colp_sw = av(SH + p * SJ + m0, list(UDsw))
        colq_sw = av(SH + q * SJ + m0, list(UDsw))
        cb = bass.AP(tensor=c16.tensor, offset=c16.offset + k * mp + cofs,
                     ap=[list(c16.ap[0]), [0, 2], [0, 8], [1, mu]])
        srb = bass.AP(tensor=sr16.tensor, offset=sr16.offset + k * mp + cofs,
                      ap=[list(sr16.ap[0]), [0, 2], [0, 8], [1, mu]])
        s2b = bass.AP(tensor=s2c.tensor, offset=s2c.offset + k * mp + cofs,
                      ap=[list(s2c.ap[0]), [4 * mp, 2], [0, 8], [1, mu]])

        def CW(tag):
            return cp.tile([128, 2, 8, mu], f16, tag=f"{tag}{mu}g{m0}",
                           name=tag)[:]

        tP, uP, tQ, uQ = CW("tP"), CW("uP"), CW("tQ"), CW("uQ")
        TT(tP, srb, colq, ALU.mult)
        TT(uP, s2b, colq_sw, ALU.mult)
        GT(tQ, srb, colp, ALU.mult)
        TT(uQ, s2b, colp_sw, ALU.mult)
        TT(colp, cb, colp, ALU.mult)
        TT(colp, colp, tP, ALU.add)
        TT(colp, colp, uP, ALU.add)
        TT(colq, cb, colq, ALU.mult)
        TT(colq, colq, tQ, ALU.subtract)
        TT(colq, colq, uQ, ALU.add)

        # Hermitian row restore (merged rows p,q): rows <- conj(cols).
        # The (p,q)/(q,p) entries race within the merged ops but are
        # explicitly zeroed below.
        dROW = [[(q - p) * SI, 2], [SJ, 8], [1, mu]]
        sCOL = [[(q - p) * SJ, 2], [SI, 8], [1, mu]]
        nc.scalar.copy(av(p * SI + m0, list(dROW)), av(p * SJ + m0, list(sCOL)))
        for rw in (p, q):
            nc.vector.tensor_scalar(
                av(SH + rw * SI + m0, [[SJ, 8], [1, mu]]),
                av(SH + rw * SJ + m0, [[SI, 8], [1, mu]]),
                -1.0, None, ALU.mult)

        # diag mirror (fp16 <- f32 Dg) + annihilated-entry zeros
        mdst = av(p * (SI + SJ) + m0, [[(q - p) * (SI + SJ), 2], [1, mu]])
        msrc = dv(p * NM + m0, [[(q - p) * NM, 2], [1, mu]])
        nc.gpsimd.tensor_copy(mdst, msrc)
        nc.scalar.memzero(av(SH + p * (SI + SJ) + m0,
                             [[(q - p) * SI, 2], [(q - p) * SJ, 2], [1, mu]]))
        nc.gpsimd.memset(av(p * SI + q * SJ + m0,
                            [[(q - p) * (SI - SJ), 2], [1, mu]]), 0.0)

    # ---- sweeps ----
    # F-part: two m-groups, params of each group's next round software-
    # pipelined into the other group's pair updates (fills the params-chain
    # latency with independent column-update work).
    GRP = ((0, 48), (48, 48))

    def pair_emitters(r, m0, mu, mp, co, cofs=0):
        return [(lambda k=k, p=p, q=q: emit_pair(k, p, q, m0, mu, mp, co, cofs))
                for k, (p, q) in enumerate(_xor_pairs(r))]

    def zip_emit(blocks, thunks):
        per = (len(thunks) + len(blocks) - 1) // len(blocks) if thunks else 0
        for i, b in enumerate(blocks):
            b()
            for t in thunks[i * per:(i + 1) * per]:
                t()

    rounds = [r for s in range(N_FULL) for r in range(1, 8)]
    co0, ops0 = make_params(rounds[0], 0, 48)
    for t in ops0:
        t()
    co1, pending1 = make_params(rounds[0], 48, 48)
    for idx, r in enumerate(rounds):
        last_pt = idx == len(rounds) - 1
        nxt = rounds[idx + 1] if idx + 1 < len(rounds) else None
        # G0 updates for round r; zip in G1's params for round r
        mu0 = MRHO if last_pt else 48
        zip_emit(pair_emitters(r, 0, mu0, 48, co0), pending1)
        # params for the next G0 round (or the first rho-only round)
        if nxt is not None:
            co0n, ops0n = make_params(nxt, 0, 48)
        else:
            co0n, ops0n = make_params(1, 0, MRHO)   # R-part round 1
        if last_pt:
            for t in ops0n:
                t()
        else:
            zip_emit(pair_emitters(r, 48, 48, 48, co1), ops0n)
            co1, pending1 = make_params(nxt, 48, 48)
        co0 = co0n

    # R-part: rho-only sweeps, two 16-matrix subgroups ping-ponged like the
    # F-part so each subgroup's params hide under the other's updates.
    rrounds = [r for s in range(N_RHO) for r in range(1, 8)]
    # co0 currently holds params emitted for R round 1 over the full rho
    # range [0, 32) (produced during the last F round) -> use for both
    # subgroups' first round via slicing-compatible mp=32 views.
    rc0 = rc1 = co0
    rmp = MRHO
    pend1 = []
    for idx, r in enumerate(rrounds):
        nxt = rrounds[idx + 1] if idx + 1 < len(rrounds) else None
        zip_emit(pair_emitters(r, 0, 16, rmp if idx == 0 else 16, rc0), pend1)
        if nxt is not None:
            rc0n, ops0n = make_params(nxt, 0, 16)
        else:
            rc0n, ops0n = None, []
        zip_emit(pair_emitters(r, 16, 16, rmp if idx == 0 else 16, rc1,
                               cofs=16 if idx == 0 else 0), ops0n)
        if nxt is not None:
            rc1n, pend1 = make_params(nxt, 16, 16)
            rc0, rc1 = rc0n, rc1n

    # ---- pt_a / pt_c diag min/max over i (final after F-part) ----
    mn = main.tile([128, 2 * NTILES], f32, name="mn")[:]
    mx = main.tile([128, 2 * NTILES], f32, name="mx")[:]
    ptv = dv(NTILES, [[1, 2 * NTILES], [NM, 8]])
    nc.vector.tensor_reduce(mn, ptv, mybir.AxisListType.X, ALU.min)
    nc.vector.tensor_reduce(mx, ptv, mybir.AxisListType.X, ALU.max)
    mu_min = mn[:, 0:NTILES]
    mu_max = mx[:, 0:NTILES]
    nu_min = mn[:, NTILES:2 * NTILES]
    nu_max = mx[:, NTILES:2 * NTILES]

    # ---- perturbative rho diag correction ----
    M = MRHO

    def Q(tag, dt=f32):
        return main.tile([128, 8, 8, M], dt, tag=tag, name=tag)[:]

    SQ, S, dif, dif2 = Q("pSQ"), Q("pS"), Q("pdif"), Q("pdif2")
    corr = main.tile([128, 8, M], f32, tag="pcorr", name="pcorr")[:]
    # |a_ij|^2 in fp32 from fp16 halves
    nc.scalar.activation(SQ, av(0, [[SI, 8], [SJ, 8], [1, M]]), ACT.Square)
    nc.scalar.activation(S, av(SH, [[SI, 8], [SJ, 8], [1, M]]), ACT.Square)
    TT(S, S, SQ, ALU.add)
    TT(dif, dv(0, [[NM, 8], [0, 8], [1, M]]),
       dv(0, [[0, 8], [NM, 8], [1, M]]), ALU.subtract)         # d_i - d_j
    nc.scalar.activation(dif2, dif, ACT.Square)
    dif2_flat = bass.AP(tensor=dif2.tensor, offset=dif2.offset,
                        ap=[list(dif2.ap[0]), [1, 64 * M]])
    nc.vector.tensor_scalar(dif2_flat, dif2_flat, PDELTA, None, ALU.add)
    nc.vector.reciprocal(dif2, dif2)                           # R
    GT(SQ, S, dif, ALU.mult)                                   # W = S*dif
    TT(SQ, SQ, dif2, ALU.mult)
    # corr_i = sum_j W[i, j, m]: reduce over j (view with j last)
    Wv = bass.AP(tensor=SQ.tensor, offset=SQ.offset,
                 ap=[list(SQ.ap[0]), [8 * M, 8], [1, M], [M, 8]])
    nc.vector.tensor_reduce(corr, Wv, mybir.AxisListType.X, ALU.add)
    TT(dv(0, [[NM, 8], [1, M]]), dv(0, [[NM, 8], [1, M]]), corr, ALU.add)

    # ---- rho diag sort (ascending), in-place 3-op comparators ----
    loc = {i: dv(i * NM, [[1, MRHO]]) for i in range(8)}
    tmin = main.tile([128, MRHO], f32, name="tmin")[:]
    for (i, j) in _CE8:
        di, dj = loc[i], loc[j]
        TT(tmin, di, dj, ALU.min)
        TT(dj, di, dj, ALU.max)
        nc.gpsimd.tensor_copy(di, tmin)

    # ---- loss assembly (fused STT forms, tree accumulation) ----
    def L(name):
        return main.tile([128, NTILES], f32, tag=name, name=name)[:]

    STTG = nc.gpsimd.scalar_tensor_tensor
    b0, b1 = L("b0"), L("b1")
    nc.vector.tensor_scalar(b0, loc[0], -8.0, 1.0, ALU.mult, ALU.add)
    nc.vector.reciprocal(b0, b0)
    nc.vector.tensor_scalar(b1, loc[7], -8.0, 1.0, ALU.mult, ALU.add)
    nc.vector.reciprocal(b1, b1)

    assert 1 <= k0 <= 8 and 1 <= k1 <= 8
    t1, t2_, acc = L("t1"), L("t2"), L("acc")
    sA, sB = L("sA"), L("sB")
    if k0 == 4:
        TT(sA, loc[0], loc[1], ALU.add)
        GT(sB, loc[2], loc[3], ALU.add)
        TT(t1, sA, sB, ALU.add)
    else:
        nc.gpsimd.tensor_copy(t1, loc[0])
        for i in range(1, k0):
            TT(t1, t1, loc[i], ALU.add)
    if k1 == 4:
        TT(sA, loc[7], loc[6], ALU.add)
        GT(sB, loc[5], loc[4], ALU.add)
        TT(t2_, sA, sB, ALU.add)
    else:
        nc.gpsimd.tensor_copy(t2_, loc[7])
        for i in range(6, 7 - k1, -1):
            TT(t2_, t2_, loc[i], ALU.add)
    u0, u1 = L("u0"), L("u1")
    STT(u0, t1, -k0 / 8.0, b0, ALU.add, ALU.mult)     # b0*(S-k0/8)
    STT(u1, t2_, -k1 / 8.0, b1, ALU.add, ALU.mult)
    TT(u0, u0, u1, ALU.add)
    nc.vector.tensor_scalar(u0, u0, (k0 + k1) / 8.0, None, ALU.add)  # l01
    TT(acc, u0, u0, ALU.mult)
    t3s = [L(f"t3{i}") for i in range(4)]
    for n, (beta, ext) in enumerate(
            ((b0, mu_min), (b1, mu_max), (b0, nu_min), (b1, nu_max))):
        t3 = t3s[n]
        STT(t3, ext, -0.125, beta, ALU.add, ALU.mult)
        nc.vector.tensor_scalar(t3, t3, 0.125, None, ALU.add)
        if n % 2 == 0:
            TT(t3, t3, t3, ALU.mult)
        else:
            GT(t3, t3, t3, ALU.mult)
    TT(t3s[0], t3s[0], t3s[1], ALU.add)
    GT(t3s[2], t3s[2], t3s[3], ALU.add)
    TT(t3s[0], t3s[0], t3s[2], ALU.add)
    TT(acc, acc, t3s[0], ALU.add)

    nc.sync.dma_start(out=out_d[:, :], in_=acc)


_prog_cache = {}


def kernel(rho_vec, rank0, rank1):
    rho_vec = np.asarray(rho_vec, dtype=np.float32)
    k0 = D - int(rank0)
    k1 = D - int(rank1)
    in_maps = _host_prep(rho_vec)

    from concourse.bass_utils import run_bass_kernel_spmd
    key = (k0, k1)
    if key not in _prog_cache:
        _prog_cache[key] = _build_program(k0, k1)
    nc = _prog_cache[key]
    res = run_bass_kernel_spmd(nc, in_maps, core_ids=list(range(NCORES)))
    return np.concatenate(
        [np.asarray(res.results[c]["out"]).T.reshape(-1) for c in range(NCORES)]
    ).astype(np.float32)


# revision 43
# speedup vs baseline: 2.3333x; 1.0034x over previous
"""Trainium2 Bass kernel for nn_BESNumEigen3qubitModel (v2).

Math reduction (exact): every eigvalsh in the reference reduces to
eigenvalues of 3 Hermitian 8x8 matrices per batch element: rho, pt_a(rho),
pt_c(rho) (see kernel_baseline for the derivation).

Device algorithm (per core: 4096 batch elems -> 128 partitions x 32 tiles,
3 matrix types -> 96 matrices per partition):
  - Matrix data fp16, layout [128, h(2), i(8), j(8), m(96)] with the matrix
    index m LAST (stride 1) so every DVE operand is packed 2-byte -> 2x/4x
    DVE throughput. Authoritative diagonal kept in f32 [128, 8, 96].
  - Cyclic complex Jacobi in XOR-pair rounds: per round the 4 pairs' rotation
    params are computed batched (exact: a round's 2x2 blocks are disjoint),
    then per-pair column update + Hermitian row restore sequentially.
  - 2 full sweeps (all 96), 1 extra rho-only sweep, then a 2nd-order
    perturbative diag correction for rho from the residual off-diagonal.
    PT matrices only need extremal eigenvalues (their final-round column
    updates are dead and skipped).
  - Sort rho diag (Batcher network), min/max-reduce PT diags, assemble loss.
"""

import numpy as np

D = 8
BATCH = 32768
NCORES = 8
PER_CORE = BATCH // NCORES       # 4096
NTILES = PER_CORE // 128         # 32 tiles per core
NM = 3 * NTILES                  # 96 matrices per partition (type-major)
MR = NM                          # full-round matrix count
MRHO = NTILES                    # rho-only count

# elem strides inside the fp16 matrix tile [2(h), 8(i), 8(j), NM(m)]
SM, SJ, SI, SH = 1, NM, 8 * NM, 64 * NM
ASIZE = 2 * 8 * 8 * NM           # 12288
PDELTA = 1e-6                    # perturbative-correction regularizer

_f32 = np.float32


# ---------------------------------------------------------------- host prep --

def _gellmann_basis(d):
    mats = []
    for j in range(d):
        for k in range(j + 1, d):
            m = np.zeros((d, d), np.complex128); m[j, k] = 1; m[k, j] = 1
            mats.append(m)
    for j in range(d):
        for k in range(j + 1, d):
            m = np.zeros((d, d), np.complex128); m[j, k] = -1j; m[k, j] = 1j
            mats.append(m)
    for l in range(1, d):
        m = np.zeros((d, d), np.complex128)
        m[np.arange(l), np.arange(l)] = 1
        m[l, l] = -l
        mats.append(np.sqrt(2.0 / (l * (l + 1))) * m)
    return np.stack(mats)


def _entry_perm(kind):
    p = np.zeros(64, np.int64)
    for i in range(8):
        for j in range(8):
            if kind == 'a':
                i2, j2 = (j & 4) | (i & 3), (i & 4) | (j & 3)
            else:
                i2, j2 = (i & 6) | (j & 1), (j & 6) | (i & 1)
            p[i * 8 + j] = i2 * 8 + j2
    return p


def _build_maps():
    """[64, 384] f32: (vec,1) -> 128 floats (f = h*64 + i*8 + j) of each of
    rho, pt_a(rho), pt_c(rho)."""
    G = _gellmann_basis(D)
    B = np.zeros((64, 128), np.float64)
    for k in range(63):
        B[k, :64] = G[k].real.reshape(-1)
        B[k, 64:] = G[k].imag.reshape(-1)
    B[63, :64] = (np.eye(D) / D).reshape(-1)

    def float_perm(kind):
        e = _entry_perm(kind)
        return np.concatenate([e, 64 + e])

    M3 = np.concatenate([B, B[:, float_perm('a')], B[:, float_perm('c')]], axis=1)
    return M3.astype(_f32)


_M3 = None


def _host_prep(rho_vec):
    global _M3
    if _M3 is None:
        _M3 = _build_maps()
    vec = rho_vec.astype(np.float64)
    vec = vec / np.linalg.norm(vec, axis=-1, keepdims=True)
    vec_aug = np.concatenate(
        [vec.astype(_f32), np.ones((vec.shape[0], 1), _f32)], axis=1)
    flat = vec_aug @ _M3                                   # [B, 384] f32
    arr = flat.reshape(NCORES, NTILES, 128, 3, 128)        # [core,tile,part,type,f]
    ins = []
    diag_f = np.array([i * 8 + i for i in range(8)])
    for c in range(NCORES):
        a = arr[c]
        # fp16 matrices: [part, f, type, tile] -> [128, f*96 + type*32 + tile]
        m16 = np.ascontiguousarray(
            a.transpose(1, 3, 2, 0).reshape(128, 128 * NM)).astype(np.float16)
        # f32 diag: [part, i, type, tile] -> [128, i*96 + m]
        dg = np.ascontiguousarray(
            a[:, :, :, diag_f].transpose(1, 3, 2, 0).reshape(128, 8 * NM)
        ).astype(_f32)
        ins.append({"mats": m16, "diag": dg})
    return ins


# ------------------------------------------------------------ device kernel --

def _xor_pairs(r):
    return [(i, i ^ r) for i in range(8) if i < (i ^ r)]


def _enum_bits(r):
    """Enumeration bit-steps (descending) for pset = {p: bit_bmax(r)(p)=0},
    enumerated in ascending-p order."""
    bmax = 4 if r >= 4 else (2 if r >= 2 else 1)
    return [b for b in (4, 2, 1) if b != bmax]


# Batcher odd-even mergesort network for 8 elements (19 comparators)
_CE8 = [(0, 1), (2, 3), (4, 5), (6, 7), (0, 2), (1, 3), (4, 6), (5, 7),
        (1, 2), (5, 6), (0, 4), (1, 5), (2, 6), (3, 7), (2, 4), (3, 5),
        (1, 2), (3, 4), (5, 6)]

N_FULL = 2      # full sweeps (all 3 matrix types)
N_RHO = 1       # extra rho-only sweeps


def _build_program(k0, k1):
    import concourse.bass as bass
    import concourse.bacc as bacc
    import concourse.mybir as mybir
    from concourse.tile import TileContext
    from contextlib import ExitStack

    f32 = mybir.dt.float32
    f16 = mybir.dt.float16
    ALU = mybir.AluOpType
    ACT = mybir.ActivationFunctionType

    nc = bacc.Bacc("TRN2")
    mats_d = nc.dram_tensor("mats", [128, ASIZE], f16, kind="ExternalInput")
    diag_d = nc.dram_tensor("diag", [128, 8 * NM], f32, kind="ExternalInput")
    out_d = nc.dram_tensor("out", [128, NTILES], f32, kind="ExternalOutput")

    with ExitStack() as ctx:
        tc = ctx.enter_context(TileContext(nc))
        main = ctx.enter_context(tc.tile_pool(name="main", bufs=1))
        pp = ctx.enter_context(tc.tile_pool(name="pp", bufs=2))
        cp = ctx.enter_context(tc.tile_pool(name="cp", bufs=2))

        A = main.tile([128, ASIZE], f16, name="A")
        Dg = main.tile([128, 8 * NM], f32, name="Dg")
        Aap = A[:]
        Dap = Dg[:]
        pdim = list(Aap.ap[0])

        def av(offset, dims):
            return bass.AP(tensor=Aap.tensor, offset=Aap.offset + offset,
                           ap=[pdim] + dims)

        def dv(offset, dims):
            return bass.AP(tensor=Dap.tensor, offset=Dap.offset + offset,
                           ap=[list(Dap.ap[0])] + dims)

        nc.sync.dma_start(out=Dg[:], in_=diag_d[:, :])
        NCHUNK = 4
        for ch in range(NCHUNK):
            w = ASIZE // NCHUNK
            nc.sync.dma_start(out=av(ch * w, [[1, w]]),
                              in_=mats_d[:, ch * w:(ch + 1) * w])

        eps30 = main.tile([128, 1], f32, name="eps30")
        nc.vector.memset(eps30[:], 1e-30)
        eps35 = main.tile([128, 1], f32, name="eps35")
        nc.vector.memset(eps35[:], 1e-35)

        with nc.allow_low_precision(reason="fp16 Jacobi data by design"):
            _emit_jacobi(nc, bass, mybir, main, pp, cp, av, dv,
                         eps30, eps35, out_d, k0, k1)

    nc.finalize()
    return nc


def _emit_jacobi(nc, bass, mybir, main, pp, cp, av, dv, eps30, eps35,
                 out_d, k0, k1):
    f32 = mybir.dt.float32
    f16 = mybir.dt.float16
    ALU = mybir.AluOpType
    ACT = mybir.ActivationFunctionType
    TT = nc.vector.tensor_tensor
    GT = nc.gpsimd.tensor_tensor
    STT = nc.vector.scalar_tensor_tensor

    def make_params(r, m0, mp):
        """Rotation params for round r, matrices m in [m0, m0+mp).
        Returns ((c16, sr16, s2c), [op thunks]) - thunks emit one op each,
        in dependency order, so callers can interleave them with other work."""
        b1, b2 = _enum_bits(r)  # descending

        def merged(dims):
            if dims[0][0] == 2 * dims[1][0]:
                return [[dims[1][0], 4]] + dims[2:]
            return dims

        sgn = lambda b: -1 if (r & b) else 1
        xdims = merged([[b1 * SI + sgn(b1) * b1 * SJ, 2],
                        [b2 * SI + sgn(b2) * b2 * SJ, 2], [1, mp]])
        Xv = av(r * SJ + m0, list(xdims))
        Yv = av(SH + r * SJ + m0, list(xdims))
        appv = dv(m0, merged([[b1 * NM, 2], [b2 * NM, 2], [1, mp]]))
        aqqv = dv(r * NM + m0, merged([[sgn(b1) * b1 * NM, 2],
                                       [sgn(b2) * b2 * NM, 2], [1, mp]]))

        def P(tag, dt=f32):
            return pp.tile([128, 4, mp], dt, tag=f"{tag}{mp}g{m0}", name=tag)[:]

        sqx, sqy, n2p, g = P("sqx"), P("sqy"), P("n2p"), P("g")
        gsq, s2, h, ag = P("gsq"), P("s2"), P("h"), P("ag")
        den, T, sg, T2 = P("den"), P("T"), P("sg"), P("T2")
        t2, cden, u, urb2 = P("t2"), P("cden"), P("u"), P("urb2")
        tb = P("tb")
        c16 = P("c16", f16)
        sr16 = P("sr16", f16)
        s2c = pp.tile([128, 2, 4, mp], f16, tag=f"s2c{mp}g{m0}", name="s2c")[:]

        ops = [
            lambda: nc.scalar.activation(sqx, Xv, ACT.Square, scale=2.0),
            lambda: nc.scalar.activation(sqy, Yv, ACT.Square, scale=2.0),
            lambda: TT(g, appv, aqqv, ALU.subtract),
            lambda: nc.scalar.activation(ag, g, ACT.Abs),
            lambda: nc.scalar.sign(sg, g, bias=eps35[:]),
            lambda: nc.scalar.activation(gsq, g, ACT.Square),
            lambda: TT(n2p, sqx, sqy, ALU.add),
            lambda: TT(s2, gsq, n2p, ALU.add),
            lambda: nc.scalar.activation(h, s2, ACT.Sqrt, bias=eps30[:]),
            lambda: GT(den, ag, h, ALU.add),
            lambda: nc.vector.reciprocal(T, den),
            lambda: GT(T2, T, T, ALU.mult),
            lambda: TT(t2, n2p, T2, ALU.mult),
            lambda: nc.scalar.activation(cden, t2, ACT.Sqrt, bias=1.0),
            lambda: nc.vector.reciprocal(c16, cden),
            lambda: GT(u, T, sg, ALU.mult),
            lambda: STT(urb2, u, 2.0, c16, ALU.mult, ALU.mult),
            lambda: TT(sr16, urb2, Xv, ALU.mult),
            lambda: TT(s2c[:, 0], urb2, Yv, ALU.mult),
            lambda: nc.vector.tensor_scalar(s2c[:, 1], s2c[:, 0], -1.0,
                                            None, ALU.mult),
            lambda: STT(tb, u, 0.5, n2p, ALU.mult, ALU.mult),
            lambda: GT(appv, appv, tb, ALU.add),
            lambda: GT(aqqv, aqqv, tb, ALU.subtract),
        ]
        return (c16, sr16, s2c), ops

    def emit_pair(k, p, q, m0, mu, mp, coeffs, cofs=0):
        """Column update + Hermitian restore for pair (p, q), m in [m0, m0+mu).
        cofs: m-offset of this update range inside the coefficient tiles."""
        c16, sr16, s2c = coeffs
        UD = [[SH, 2], [SI, 8], [1, mu]]
        UDsw = [[-SH, 2], [SI, 8], [1, mu]]
        colp = av(p * SJ + m0, list(UD))
        colq = av(q * SJ + m0, list(UD))
        colp_sw = av(SH + p * SJ + m0, list(UDsw))
        colq_sw = av(SH + q * SJ + m0, list(UDsw))
        cb = bass.AP(tensor=c16.tensor, offset=c16.offset + k * mp + cofs,
                     ap=[list(c16.ap[0]), [0, 2], [0, 8], [1, mu]])
        srb = bass.AP(tensor=sr16.tensor, offset=sr16.offset + k * mp + cofs,
                      ap=[list(sr16.ap[0]), [0, 2], [0, 8], [1, mu]])
        s2b = bass.AP(tensor=s2c.tensor, offset=s2c.offset + k * mp + cofs,
                      ap=[list(s2c.ap[0]), [4 * mp, 2], [0, 8], [1, mu]])

        def CW(tag):
            return cp.tile([128, 2, 8, mu], f16, tag=f"{tag}{mu}g{m0}",
                           name=tag)[:]

        tP, uP, tQ, uQ = CW("tP"), CW("uP"), CW("tQ"), CW("uQ")
        TT(tP, srb, colq, ALU.mult)
        TT(uP, s2b, colq_sw, ALU.mult)
        GT(tQ, srb, colp, ALU.mult)
        TT(uQ, s2b, colp_sw, ALU.mult)
        TT(colp, cb, colp, ALU.mult)
        TT(colp, colp, tP, ALU.add)
        TT(colp, colp, uP, ALU.add)
        TT(colq, cb, colq, ALU.mult)
        TT(colq, colq, tQ, ALU.subtract)
        TT(colq, colq, uQ, ALU.add)

        # Hermitian row restore (merged rows p,q): rows <- conj(cols).
        # The (p,q)/(q,p) entries race within the merged ops but are
        # explicitly zeroed below.
        dROW = [[(q - p) * SI, 2], [SJ, 8], [1, mu]]
        sCOL = [[(q - p) * SJ, 2], [SI, 8], [1, mu]]
        nc.scalar.copy(av(p * SI + m0, list(dROW)), av(p * SJ + m0, list(sCOL)))
        for rw in (p, q):
            nc.vector.tensor_scalar(
                av(SH + rw * SI + m0, [[SJ, 8], [1, mu]]),
                av(SH + rw * SJ + m0, [[SI, 8], [1, mu]]),
                -1.0, None, ALU.mult)

        # diag mirror (fp16 <- f32 Dg) + annihilated-entry zeros
        mdst = av(p * (SI + SJ) + m0, [[(q - p) * (SI + SJ), 2], [1, mu]])
        msrc = dv(p * NM + m0, [[(q - p) * NM, 2], [1, mu]])
        nc.gpsimd.tensor_copy(mdst, msrc)
        nc.scalar.memzero(av(SH + p * (SI + SJ) + m0,
                             [[(q - p) * SI, 2], [(q - p) * SJ, 2], [1, mu]]))
        nc.gpsimd.memset(av(p * SI + q * SJ + m0,
                            [[(q - p) * (SI - SJ), 2], [1, mu]]), 0.0)

    # ---- sweeps ----
    # F-part: two m-groups, params of each group's next round software-
    # pipelined into the other group's pair updates (fills the params-chain
    # latency with independent column-update work).
    GRP = ((0, 48), (48, 48))

    def pair_emitters(r, m0, mu, mp, co, cofs=0):
        return [(lambda k=k, p=p, q=q: emit_pair(k, p, q, m0, mu, mp, co, cofs))
                for k, (p, q) in enumerate(_xor_pairs(r))]

    def zip_emit(blocks, thunks):
        per = (len(thunks) + len(blocks) - 1) // len(blocks) if thunks else 0
        for i, b in enumerate(blocks):
            b()
            for t in thunks[i * per:(i + 1) * per]:
                t()

    rounds = [r for s in range(N_FULL) for r in range(1, 8)]
    co0, ops0 = make_params(rounds[0], 0, 48)
    for t in ops0:
        t()
    co1, pending1 = make_params(rounds[0], 48, 48)
    for idx, r in enumerate(rounds):
        last_pt = idx == len(rounds) - 1
        nxt = rounds[idx + 1] if idx + 1 < len(rounds) else None
        # G0 updates for round r; zip in G1's params for round r
        mu0 = MRHO if last_pt else 48
        zip_emit(pair_emitters(r, 0, mu0, 48, co0), pending1)
        # params for the next G0 round (or the first rho-only round)
        if nxt is not None:
            co0n, ops0n = make_params(nxt, 0, 48)
        else:
            co0n, ops0n = make_params(1, 0, MRHO)   # R-part round 1
        if last_pt:
            for t in ops0n:
                t()
        else:
            zip_emit(pair_emitters(r, 48, 48, 48, co1), ops0n)
            co1, pending1 = make_params(nxt, 48, 48)
        co0 = co0n

    # R-part: rho-only sweeps, two 16-matrix subgroups ping-ponged like the
    # F-part so each subgroup's params hide under the other's updates.
    rrounds = [r for s in range(N_RHO) for r in range(1, 8)]
    # co0 currently holds params emitted for R round 1 over the full rho
    # range [0, 32) (produced during the last F round) -> use for both
    # subgroups' first round via slicing-compatible mp=32 views.
    rc0 = rc1 = co0
    rmp = MRHO
    pend1 = []
    for idx, r in enumerate(rrounds):
        nxt = rrounds[idx + 1] if idx + 1 < len(rrounds) else None
        zip_emit(pair_emitters(r, 0, 16, rmp if idx == 0 else 16, rc0), pend1)
        if nxt is not None:
            rc0n, ops0n = make_params(nxt, 0, 16)
        else:
            rc0n, ops0n = None, []
        zip_emit(pair_emitters(r, 16, 16, rmp if idx == 0 else 16, rc1,
                               cofs=16 if idx == 0 else 0), ops0n)
        if nxt is not None:
            rc1n, pend1 = make_params(nxt, 16, 16)
            rc0, rc1 = rc0n, rc1n

    # ---- pt_a / pt_c diag min/max over i (final after F-part) ----
    mn = main.tile([128, 2 * NTILES], f32, name="mn")[:]
    mx = main.tile([128, 2 * NTILES], f32, name="mx")[:]
    ptv = dv(NTILES, [[1, 2 * NTILES], [NM, 8]])
    nc.vector.tensor_reduce(mn, ptv, mybir.AxisListType.X, ALU.min)
    nc.vector.tensor_reduce(mx, ptv, mybir.AxisListType.X, ALU.max)
    mu_min = mn[:, 0:NTILES]
    mu_max = mx[:, 0:NTILES]
    nu_min = mn[:, NTILES:2 * NTILES]
    nu_max = mx[:, NTILES:2 * NTILES]

    # ---- perturbative rho diag correction ----
    M = MRHO

    def Q(tag, dt=f32):
        return main.tile([128, 8, 8, M], dt, tag=tag, name=tag)[:]

    SQ, S, dif, dif2 = Q("pSQ"), Q("pS"), Q("pdif"), Q("pdif2")
    corr = main.tile([128, 8, M], f32, tag="pcorr", name="pcorr")[:]
    # |a_ij|^2 in fp32 from fp16 halves
    nc.scalar.activation(SQ, av(0, [[SI, 8], [SJ, 8], [1, M]]), ACT.Square)
    nc.scalar.activation(S, av(SH, [[SI, 8], [SJ, 8], [1, M]]), ACT.Square)
    TT(S, S, SQ, ALU.add)
    TT(dif, dv(0, [[NM, 8], [0, 8], [1, M]]),
       dv(0, [[0, 8], [NM, 8], [1, M]]), ALU.subtract)         # d_i - d_j
    nc.scalar.activation(dif2, dif, ACT.Square)
    dif2_flat = bass.AP(tensor=dif2.tensor, offset=dif2.offset,
                        ap=[list(dif2.ap[0]), [1, 64 * M]])
    nc.vector.tensor_scalar(dif2_flat, dif2_flat, PDELTA, None, ALU.add)
    nc.vector.reciprocal(dif2, dif2)                           # R
    TT(SQ, S, dif, ALU.mult)                                   # W = S*dif
    TT(SQ, SQ, dif2, ALU.mult)
    # corr_i = sum_j W[i, j, m]: reduce over j (view with j last)
    Wv = bass.AP(tensor=SQ.tensor, offset=SQ.offset,
                 ap=[list(SQ.ap[0]), [8 * M, 8], [1, M], [M, 8]])
    nc.vector.tensor_reduce(corr, Wv, mybir.AxisListType.X, ALU.add)
    TT(dv(0, [[NM, 8], [1, M]]), dv(0, [[NM, 8], [1, M]]), corr, ALU.add)

    # ---- rho diag sort (ascending), in-place 3-op comparators ----
    loc = {i: dv(i * NM, [[1, MRHO]]) for i in range(8)}
    tmin = main.tile([128, MRHO], f32, name="tmin")[:]
    # with k0 == k1 == 4 the final intra-half orderings (1,2), (5,6) are
    # irrelevant: S_4/T_4 are sums and only the 4/4 split + extremes matter
    ce = _CE8 if not (k0 == 4 and k1 == 4) else (_CE8[:16] + [(3, 4)])
    for (i, j) in ce:
        di, dj = loc[i], loc[j]
        TT(tmin, di, dj, ALU.min)
        TT(dj, di, dj, ALU.max)
        nc.gpsimd.tensor_copy(di, tmin)

    # ---- loss assembly (fused STT forms, tree accumulation) ----
    def L(name):
        return main.tile([128, NTILES], f32, tag=name, name=name)[:]

    STTG = nc.gpsimd.scalar_tensor_tensor
    b0, b1 = L("b0"), L("b1")
    nc.vector.tensor_scalar(b0, loc[0], -8.0, 1.0, ALU.mult, ALU.add)
    nc.vector.reciprocal(b0, b0)
    nc.vector.tensor_scalar(b1, loc[7], -8.0, 1.0, ALU.mult, ALU.add)
    nc.vector.reciprocal(b1, b1)

    assert 1 <= k0 <= 8 and 1 <= k1 <= 8
    t1, t2_, acc = L("t1"), L("t2"), L("acc")
    sA, sB = L("sA"), L("sB")
    if k0 == 4:
        TT(sA, loc[0], loc[1], ALU.add)
        GT(sB, loc[2], loc[3], ALU.add)
        TT(t1, sA, sB, ALU.add)
    else:
        nc.gpsimd.tensor_copy(t1, loc[0])
        for i in range(1, k0):
            TT(t1, t1, loc[i], ALU.add)
    if k1 == 4:
        TT(sA, loc[7], loc[6], ALU.add)
        GT(sB, loc[5], loc[4], ALU.add)
        TT(t2_, sA, sB, ALU.add)
    else:
        nc.gpsimd.tensor_copy(t2_, loc[7])
        for i in range(6, 7 - k1, -1):
            TT(t2_, t2_, loc[i], ALU.add)
    u0, u1 = L("u0"), L("u1")
    STT(u0, t1, -k0 / 8.0, b0, ALU.add, ALU.mult)     # b0*(S-k0/8)
    STT(u1, t2_, -k1 / 8.0, b1, ALU.add, ALU.mult)
    TT(u0, u0, u1, ALU.add)
    nc.vector.tensor_scalar(u0, u0, (k0 + k1) / 8.0, None, ALU.add)  # l01
    TT(acc, u0, u0, ALU.mult)
    t3s = [L(f"t3{i}") for i in range(4)]
    for n, (beta, ext) in enumerate(
            ((b0, mu_min), (b1, mu_max), (b0, nu_min), (b1, nu_max))):
        t3 = t3s[n]
        STT(t3, ext, -0.125, beta, ALU.add, ALU.mult)
        nc.vector.tensor_scalar(t3, t3, 0.125, None, ALU.add)
        if n % 2 == 0:
            TT(t3, t3, t3, ALU.mult)
        else:
            GT(t3, t3, t3, ALU.mult)
    TT(t3s[0], t3s[0], t3s[1], ALU.add)
    GT(t3s[2], t3s[2], t3s[3], ALU.add)
    TT(t3s[0], t3s[0], t3s[2], ALU.add)
    TT(acc, acc, t3s[0], ALU.add)

    nc.sync.dma_start(out=out_d[:, :], in_=acc)


_prog_cache = {}


def kernel(rho_vec, rank0, rank1):
    rho_vec = np.asarray(rho_vec, dtype=np.float32)
    k0 = D - int(rank0)
    k1 = D - int(rank1)
    in_maps = _host_prep(rho_vec)

    from concourse.bass_utils import run_bass_kernel_spmd
    key = (k0, k1)
    if key not in _prog_cache:
        _prog_cache[key] = _build_program(k0, k1)
    nc = _prog_cache[key]
    res = run_bass_kernel_spmd(nc, in_maps, core_ids=list(range(NCORES)))
    return np.concatenate(
        [np.asarray(res.results[c]["out"]).T.reshape(-1) for c in range(NCORES)]
    ).astype(np.float32)


# revision 73
# speedup vs baseline: 2.4596x; 1.0541x over previous
"""Trainium2 Bass kernel for nn_BESNumEigen3qubitModel (v3).

Math reduction (exact): dm0/dm1 and their partial transposes are affine in
rho with the identity fixed, so every eigvalsh in the reference reduces to
eigenvalues of 3 Hermitian 8x8 matrices per batch element: rho, pt_a(rho),
pt_c(rho). With w = eig(rho) ascending, S_k0 = sum of k0 smallest, T_k1 =
sum of k1 largest, mu/nu = eig extrema of pt_a/pt_c:
   beta0 = 1/(1-8 w_min), beta1 = 1/(1-8 w_max)
   loss  = (beta0*(S_k0-k0/8)+k0/8 + beta1*(T_k1-k1/8)+k1/8)^2
           + sum over 4 PPT terms (beta*(ext-1/8)+1/8)^2.

Device algorithm (per core: 4096 batch elems -> 128 partitions x 32 tiles,
3 matrix types -> 96 matrices per partition):
  - Matrix data fp16, layout [128, h(2), i(8), j(8), m(96)] with the matrix
    index m LAST (stride 1) so every DVE operand is packed 2-byte ->
    2x (TensorTensor) / 4x (TensorCopy/TensorScalar) DVE throughput.
    The authoritative diagonal is kept in f32 [128, 8(i), 96(m)]
    (eigenvalues accumulate there at full precision).
  - Cyclic complex Jacobi in XOR-pair rounds (pairs (p, p^r), r = 1..7).
    Per round the 4 pairs' rotation params are computed batched over a
    [128, 4, m] layout (exact: a round's 2x2 pivot blocks are mutually
    disjoint); XOR-pair index sets are affine, so they are plain strided
    views. Then per-pair: 10-op fp16 column update, merged Hermitian row
    restore (rows <- conj(cols); the two racy entries are re-zeroed),
    fp16 diag mirror + annihilated-entry zeros.
  - Two independent round streams interleaved 3:2 with each stream's
    next-round params software-pipelined into the other stream's column
    updates: stream A = rho (21 rounds = 2 full + 1 extra sweep),
    stream B = pt_a/pt_c (14 rounds = 2 sweeps, final round params-only
    since only the f32 diag is read afterwards).
  - Round-1 pivot entries are DMA-prefetched into side tiles so the first
    params chain overlaps the bulk matrix DMA.
  - Tail: 2nd-order perturbative diag correction for rho from the residual
    off-diagonal (corr_i = sum_j |a_ij|^2 (d_i-d_j)/((d_i-d_j)^2+delta)),
    Batcher sort of the rho diag, min/max-reduce of PT diags, loss
    assembly - all split into two m-halves for chain overlap.
Accuracy (vs f64 reference, whole batch): max rel err ~5.3e-3.
"""

import numpy as np

D = 8
BATCH = 32768
NCORES = 8
PER_CORE = BATCH // NCORES       # 4096
NTILES = PER_CORE // 128         # 32 tiles per core
NM = 3 * NTILES                  # 96 matrices per partition (type-major)
MRHO = NTILES                    # rho-only count

# elem strides inside the fp16 matrix tile [2(h), 8(i), 8(j), NM(m)]
SM, SJ, SI, SH = 1, NM, 8 * NM, 64 * NM
ASIZE = 2 * 8 * 8 * NM           # 12288
PDELTA = 1e-6                    # perturbative-correction regularizer

_f32 = np.float32


# ---------------------------------------------------------------- host prep --

def _gellmann_basis(d):
    mats = []
    for j in range(d):
        for k in range(j + 1, d):
            m = np.zeros((d, d), np.complex128); m[j, k] = 1; m[k, j] = 1
            mats.append(m)
    for j in range(d):
        for k in range(j + 1, d):
            m = np.zeros((d, d), np.complex128); m[j, k] = -1j; m[k, j] = 1j
            mats.append(m)
    for l in range(1, d):
        m = np.zeros((d, d), np.complex128)
        m[np.arange(l), np.arange(l)] = 1
        m[l, l] = -l
        mats.append(np.sqrt(2.0 / (l * (l + 1))) * m)
    return np.stack(mats)


def _entry_perm(kind):
    p = np.zeros(64, np.int64)
    for i in range(8):
        for j in range(8):
            if kind == 'a':
                i2, j2 = (j & 4) | (i & 3), (i & 4) | (j & 3)
            else:
                i2, j2 = (i & 6) | (j & 1), (j & 6) | (i & 1)
            p[i * 8 + j] = i2 * 8 + j2
    return p


def _build_maps():
    """[64, 384] f32: (vec,1) -> 128 floats (f = h*64 + i*8 + j) of each of
    rho, pt_a(rho), pt_c(rho)."""
    G = _gellmann_basis(D)
    B = np.zeros((64, 128), np.float64)
    for k in range(63):
        B[k, :64] = G[k].real.reshape(-1)
        B[k, 64:] = G[k].imag.reshape(-1)
    B[63, :64] = (np.eye(D) / D).reshape(-1)

    def float_perm(kind):
        e = _entry_perm(kind)
        return np.concatenate([e, 64 + e])

    M3 = np.concatenate([B, B[:, float_perm('a')], B[:, float_perm('c')]], axis=1)
    return M3.astype(_f32)


_M3 = None


def _host_prep(rho_vec):
    global _M3
    if _M3 is None:
        _M3 = _build_maps()
    vec = rho_vec.astype(np.float64)
    vec = vec / np.linalg.norm(vec, axis=-1, keepdims=True)
    vec_aug = np.concatenate(
        [vec.astype(_f32), np.ones((vec.shape[0], 1), _f32)], axis=1)
    flat = vec_aug @ _M3                                   # [B, 384] f32
    arr = flat.reshape(NCORES, NTILES, 128, 3, 128)        # [core,tile,part,type,f]
    ins = []
    diag_f = np.array([i * 8 + i for i in range(8)])
    for c in range(NCORES):
        a = arr[c]
        # fp16 matrices: [part, f, type, tile] -> [128, f*96 + type*32 + tile]
        m16 = np.ascontiguousarray(
            a.transpose(1, 3, 2, 0).reshape(128, 128 * NM)).astype(np.float16)
        # f32 diag: [part, i, type, tile] -> [128, i*96 + m]
        dg = np.ascontiguousarray(
            a[:, :, :, diag_f].transpose(1, 3, 2, 0).reshape(128, 8 * NM)
        ).astype(_f32)
        ins.append({"mats": m16, "diag": dg})
    return ins


# ------------------------------------------------------------ device kernel --

def _xor_pairs(r):
    return [(i, i ^ r) for i in range(8) if i < (i ^ r)]


def _enum_bits(r):
    """Enumeration bit-steps (descending) for pset = {p: bit_bmax(r)(p)=0},
    enumerated in ascending-p order."""
    bmax = 4 if r >= 4 else (2 if r >= 2 else 1)
    return [b for b in (4, 2, 1) if b != bmax]


# Batcher odd-even mergesort network for 8 elements (19 comparators)
_CE8 = [(0, 1), (2, 3), (4, 5), (6, 7), (0, 2), (1, 3), (4, 6), (5, 7),
        (1, 2), (5, 6), (0, 4), (1, 5), (2, 6), (3, 7), (2, 4), (3, 5),
        (1, 2), (3, 4), (5, 6)]

N_FULL = 2      # full sweeps (all 3 matrix types)
N_RHO = 1       # extra rho-only sweeps


def _build_program(k0, k1):
    import concourse.bass as bass
    import concourse.bacc as bacc
    import concourse.mybir as mybir
    from concourse.tile import TileContext
    from contextlib import ExitStack

    f32 = mybir.dt.float32
    f16 = mybir.dt.float16
    ALU = mybir.AluOpType
    ACT = mybir.ActivationFunctionType

    nc = bacc.Bacc("TRN2")
    mats_d = nc.dram_tensor("mats", [128, ASIZE], f16, kind="ExternalInput")
    diag_d = nc.dram_tensor("diag", [128, 8 * NM], f32, kind="ExternalInput")
    out_d = nc.dram_tensor("out", [128, NTILES], f32, kind="ExternalOutput")

    with ExitStack() as ctx:
        tc = ctx.enter_context(TileContext(nc))
        main = ctx.enter_context(tc.tile_pool(name="main", bufs=1))
        pp = ctx.enter_context(tc.tile_pool(name="pp", bufs=2))
        cp = ctx.enter_context(tc.tile_pool(name="cp", bufs=2))

        A = main.tile([128, ASIZE], f16, name="A")
        Dg = main.tile([128, 8 * NM], f32, name="Dg")
        Aap = A[:]
        Dap = Dg[:]
        pdim = list(Aap.ap[0])

        def av(offset, dims):
            return bass.AP(tensor=Aap.tensor, offset=Aap.offset + offset,
                           ap=[pdim] + dims)

        def dv(offset, dims):
            return bass.AP(tensor=Dap.tensor, offset=Dap.offset + offset,
                           ap=[list(Dap.ap[0])] + dims)

        nc.sync.dma_start(out=Dg[:], in_=diag_d[:, :])
        # Prefetch round-1 pivot entries (f = p*8+q and the im half) on the
        # Act engine's DMA queue so the first params chain fully overlaps the
        # bulk matrix DMA below. r=1 pairs (0,1),(2,3),(4,5),(6,7) -> f = 1,
        # 19, 37, 55 (re) and +64 (im), each an 18-step affine f-row set.
        xpre = main.tile([128, 4 * NM], f16, name="xpre")
        ypre = main.tile([128, 4 * NM], f16, name="ypre")
        md = mats_d[:, :]

        def mrows(foff):
            return bass.AP(tensor=md.tensor, offset=md.offset + foff * NM,
                           ap=[list(md.ap[0]), [18 * NM, 4], [1, NM]])

        nc.sync.dma_start(out=xpre[:], in_=mrows(1))
        nc.sync.dma_start(out=ypre[:], in_=mrows(65))
        NCHUNK = 2
        for ch in range(NCHUNK):
            w = ASIZE // NCHUNK
            nc.sync.dma_start(out=av(ch * w, [[1, w]]),
                              in_=mats_d[:, ch * w:(ch + 1) * w])

        eps30 = main.tile([128, 1], f32, name="eps30")
        nc.vector.memset(eps30[:], 1e-30)
        eps35 = main.tile([128, 1], f32, name="eps35")
        nc.vector.memset(eps35[:], 1e-35)

        with nc.allow_low_precision(reason="fp16 Jacobi data by design"):
            _emit_jacobi(nc, bass, mybir, main, pp, cp, av, dv,
                         eps30, eps35, out_d, k0, k1, xpre, ypre)

    nc.finalize()
    return nc


def _emit_jacobi(nc, bass, mybir, main, pp, cp, av, dv, eps30, eps35,
                 out_d, k0, k1, xpre, ypre):
    f32 = mybir.dt.float32
    f16 = mybir.dt.float16
    ALU = mybir.AluOpType
    ACT = mybir.ActivationFunctionType
    TT = nc.vector.tensor_tensor
    GT = nc.gpsimd.tensor_tensor
    STT = nc.vector.scalar_tensor_tensor

    def make_params(r, m0, mp, xy_override=None):
        """Rotation params for round r, matrices m in [m0, m0+mp).
        Returns ((c16, sr16, s2c), [op thunks]) - thunks emit one op each,
        in dependency order, so callers can interleave them with other work.
        xy_override: (Xv, Yv) APs to read the pivot entries from instead of
        the matrix tile (used for the DMA-prefetched first round)."""
        b1, b2 = _enum_bits(r)  # descending

        def merged(dims):
            if dims[0][0] == 2 * dims[1][0]:
                return [[dims[1][0], 4]] + dims[2:]
            return dims

        sgn = lambda b: -1 if (r & b) else 1
        xdims = merged([[b1 * SI + sgn(b1) * b1 * SJ, 2],
                        [b2 * SI + sgn(b2) * b2 * SJ, 2], [1, mp]])
        if xy_override is not None:
            Xv, Yv = xy_override
        else:
            Xv = av(r * SJ + m0, list(xdims))
            Yv = av(SH + r * SJ + m0, list(xdims))
        appv = dv(m0, merged([[b1 * NM, 2], [b2 * NM, 2], [1, mp]]))
        aqqv = dv(r * NM + m0, merged([[sgn(b1) * b1 * NM, 2],
                                       [sgn(b2) * b2 * NM, 2], [1, mp]]))

        def P(tag, dt=f32):
            return pp.tile([128, 4, mp], dt, tag=f"{tag}{mp}g{m0}", name=tag)[:]

        sqx, sqy, n2p, g = P("sqx"), P("sqy"), P("n2p"), P("g")
        gsq, s2, h, ag = P("gsq"), P("s2"), P("h"), P("ag")
        den, T, sg, T2 = P("den"), P("T"), P("sg"), P("T2")
        t2, cden, u, urb2 = P("t2"), P("cden"), P("u"), P("urb2")
        tb = P("tb")
        c16 = P("c16", f16)
        sr16 = P("sr16", f16)
        s2c = pp.tile([128, 2, 4, mp], f16, tag=f"s2c{mp}g{m0}", name="s2c")[:]

        ops = [
            lambda: nc.scalar.activation(sqx, Xv, ACT.Square, scale=2.0),
            lambda: nc.scalar.activation(sqy, Yv, ACT.Square, scale=2.0),
            lambda: TT(g, appv, aqqv, ALU.subtract),
            lambda: nc.scalar.activation(ag, g, ACT.Abs),
            lambda: nc.scalar.sign(sg, g, bias=eps35[:]),
            lambda: nc.scalar.activation(gsq, g, ACT.Square),
            lambda: TT(n2p, sqx, sqy, ALU.add),
            lambda: TT(s2, gsq, n2p, ALU.add),
            lambda: nc.scalar.activation(h, s2, ACT.Sqrt, bias=eps30[:]),
            lambda: GT(den, ag, h, ALU.add),
            lambda: nc.vector.reciprocal(T, den),
            lambda: GT(T2, T, T, ALU.mult),
            lambda: TT(t2, n2p, T2, ALU.mult),
            lambda: nc.scalar.activation(cden, t2, ACT.Sqrt, bias=1.0),
            lambda: nc.vector.reciprocal(c16, cden),
            lambda: GT(u, T, sg, ALU.mult),
            lambda: STT(urb2, u, 2.0, c16, ALU.mult, ALU.mult),
            lambda: TT(sr16, urb2, Xv, ALU.mult),
            lambda: TT(s2c[:, 0], urb2, Yv, ALU.mult),
            lambda: nc.scalar.activation(s2c[:, 1], s2c[:, 0], ACT.Copy,
                                         scale=-1.0),
            lambda: STT(tb, u, 0.5, n2p, ALU.mult, ALU.mult),
            lambda: GT(appv, appv, tb, ALU.add),
            lambda: GT(aqqv, aqqv, tb, ALU.subtract),
        ]
        return (c16, sr16, s2c), ops

    def emit_pair(k, p, q, m0, mu, mp, coeffs, cofs=0):
        """Column update + Hermitian restore for pair (p, q), m in [m0, m0+mu).
        cofs: m-offset of this update range inside the coefficient tiles."""
        c16, sr16, s2c = coeffs
        UD = [[SH, 2], [SI, 8], [1, mu]]
        UDsw = [[-SH, 2], [SI, 8], [1, mu]]
        colp = av(p * SJ + m0, list(UD))
        colq = av(q * SJ + m0, list(UD))
        colp_sw = av(SH + p * SJ + m0, list(UDsw))
        colq_sw = av(SH + q * SJ + m0, list(UDsw))
        cb = bass.AP(tensor=c16.tensor, offset=c16.offset + k * mp + cofs,
                     ap=[list(c16.ap[0]), [0, 2], [0, 8], [1, mu]])
        srb = bass.AP(tensor=sr16.tensor, offset=sr16.offset + k * mp + cofs,
                      ap=[list(sr16.ap[0]), [0, 2], [0, 8], [1, mu]])
        s2b = bass.AP(tensor=s2c.tensor, offset=s2c.offset + k * mp + cofs,
                      ap=[list(s2c.ap[0]), [4 * mp, 2], [0, 8], [1, mu]])

        def CW(tag):
            return cp.tile([128, 2, 8, mu], f16, tag=f"{tag}{mu}g{m0}",
                           name=tag)[:]

        tP, uP, tQ, uQ = CW("tP"), CW("uP"), CW("tQ"), CW("uQ")
        TT(tP, srb, colq, ALU.mult)
        TT(uP, s2b, colq_sw, ALU.mult)
        GT(tQ, srb, colp, ALU.mult)
        TT(uQ, s2b, colp_sw, ALU.mult)
        TT(colp, cb, colp, ALU.mult)
        TT(colp, colp, tP, ALU.add)
        TT(colp, colp, uP, ALU.add)
        TT(colq, cb, colq, ALU.mult)
        TT(colq, colq, tQ, ALU.subtract)
        TT(colq, colq, uQ, ALU.add)

        # Hermitian row restore (merged rows p,q): rows <- conj(cols).
        # The (p,q)/(q,p) entries race within the merged ops but are
        # explicitly zeroed below.
        dROW = [[(q - p) * SI, 2], [SJ, 8], [1, mu]]
        sCOL = [[(q - p) * SJ, 2], [SI, 8], [1, mu]]
        nc.scalar.copy(av(p * SI + m0, list(dROW)), av(p * SJ + m0, list(sCOL)))
        for rw in (p, q):
            nc.vector.tensor_scalar(
                av(SH + rw * SI + m0, [[SJ, 8], [1, mu]]),
                av(SH + rw * SJ + m0, [[SI, 8], [1, mu]]),
                -1.0, None, ALU.mult)

        # diag mirror (fp16 <- f32 Dg) + annihilated-entry zeros
        mdst = av(p * (SI + SJ) + m0, [[(q - p) * (SI + SJ), 2], [1, mu]])
        msrc = dv(p * NM + m0, [[(q - p) * NM, 2], [1, mu]])
        nc.gpsimd.tensor_copy(mdst, msrc)
        nc.scalar.memzero(av(SH + p * (SI + SJ) + m0,
                             [[(q - p) * SI, 2], [(q - p) * SJ, 2], [1, mu]]))
        nc.gpsimd.memset(av(p * SI + q * SJ + m0,
                            [[(q - p) * (SI - SJ), 2], [1, mu]]), 0.0)

    # ---- sweeps: two type-aligned streams, interleaved 3:2 ----
    # Stream A: rho (m 0..31), N_FULL + N_RHO sweeps = 21 rounds (its extra
    # rho-only sweeps overlap the PT stream instead of trailing serially).
    # Stream B: pt_a/pt_c (m 32..95), N_FULL sweeps = 14 rounds; the last
    # round's column updates are dead (only the f32 diag is read afterwards)
    # and are skipped.
    # Each stream's next-round params are software-pipelined into the other
    # stream's pair updates (zip_emit) to hide the params dependency chain.

    def pair_emitters(r, m0, mu, mp, co, cofs=0):
        return [(lambda k=k, p=p, q=q: emit_pair(k, p, q, m0, mu, mp, co, cofs))
                for k, (p, q) in enumerate(_xor_pairs(r))]

    def zip_emit(blocks, thunks):
        per = (len(thunks) + len(blocks) - 1) // len(blocks) if thunks else 0
        for i, b in enumerate(blocks):
            b()
            for t in thunks[i * per:(i + 1) * per]:
                t()

    A_ROUNDS = [r for s in range(N_FULL + N_RHO) for r in range(1, 8)]
    B_ROUNDS = [r for s in range(N_FULL) for r in range(1, 8)]
    AM0, AMP = 0, MRHO          # rho range
    BM0, BMP = MRHO, NM - MRHO  # pt range

    state = {
        "A": {"rounds": A_ROUNDS, "idx": 0, "m0": AM0, "mp": AMP,
              "co": None, "pend": None},
        "B": {"rounds": B_ROUNDS, "idx": 0, "m0": BM0, "mp": BMP,
              "co": None, "pend": None},
    }
    for nm in ("A", "B"):
        st = state[nm]
        xo = bass.AP(tensor=xpre[:].tensor,
                     offset=xpre[:].offset + st["m0"],
                     ap=[list(xpre[:].ap[0]), [NM, 4], [1, st["mp"]]])
        yo = bass.AP(tensor=ypre[:].tensor,
                     offset=ypre[:].offset + st["m0"],
                     ap=[list(ypre[:].ap[0]), [NM, 4], [1, st["mp"]]])
        st["co"], st["pend"] = make_params(st["rounds"][0], st["m0"], st["mp"],
                                           xy_override=(xo, yo))

    def do_slot(x):
        other = "B" if x == "A" else "A"
        st, so = state[x], state[other]
        i = st["idx"]
        r = st["rounds"][i]
        if st["pend"]:                      # own params not yet emitted
            for t in st["pend"]:
                t()
            st["pend"] = None
        last = i == len(st["rounds"]) - 1
        dead = last and x == "B"            # pt final col updates are dead
        if not dead:
            blocks = pair_emitters(r, st["m0"], st["mp"], st["mp"], st["co"])
            zip_emit(blocks, so["pend"] or [])
            so["pend"] = None
        st["idx"] = i + 1
        if not last:
            st["co"], st["pend"] = make_params(st["rounds"][i + 1],
                                               st["m0"], st["mp"])

    # 3:2 interleave: [A B A B A] x 7 covers 21 A-rounds and 14 B-rounds
    for step in range(7):
        for x in ("A", "A", "B", "A", "B"):
            do_slot(x)
    assert state["A"]["idx"] == len(A_ROUNDS)
    assert state["B"]["idx"] == len(B_ROUNDS)

    # ---- pt_a / pt_c diag min/max over i (final after B-stream) ----
    mn = main.tile([128, 2 * NTILES], f32, name="mn")[:]
    mx = main.tile([128, 2 * NTILES], f32, name="mx")[:]
    ptv = dv(NTILES, [[1, 2 * NTILES], [NM, 8]])
    nc.vector.tensor_reduce(mn, ptv, mybir.AxisListType.X, ALU.min)
    nc.vector.tensor_reduce(mx, ptv, mybir.AxisListType.X, ALU.max)

    # ---- tail (perturbative correction, sort, loss assembly) ----
    # Split into two independent m-halves so the two dependency chains
    # overlap; everything here is elementwise per (partition, m).
    acc_full = main.tile([128, NTILES], f32, tag="acc", name="acc")[:]
    HM = MRHO // 2

    def Q(tag, dt=f32):
        return main.tile([128, 8, 8, HM], dt, tag=tag, name=tag)[:]

    halves = []
    for hx, h0 in enumerate((0, HM)):
        SQ, S, dif, dif2 = Q(f"pSQ{hx}"), Q(f"pS{hx}"), Q(f"pdif{hx}"), Q(f"pdif2{hx}")
        corr = main.tile([128, 8, HM], f32, tag=f"pcorr{hx}", name="pcorr")[:]
        halves.append((h0, SQ, S, dif, dif2, corr))

    def emit_perturb(hx):
        h0, SQ, S, dif, dif2, corr = halves[hx]
        M = HM
        nc.scalar.activation(SQ, av(h0, [[SI, 8], [SJ, 8], [1, M]]), ACT.Square)
        nc.scalar.activation(S, av(SH + h0, [[SI, 8], [SJ, 8], [1, M]]),
                             ACT.Square)
        TT(S, S, SQ, ALU.add)
        TT(dif, dv(h0, [[NM, 8], [0, 8], [1, M]]),
           dv(h0, [[0, 8], [NM, 8], [1, M]]), ALU.subtract)    # d_i - d_j
        nc.scalar.activation(dif2, dif, ACT.Square)
        dif2_flat = bass.AP(tensor=dif2.tensor, offset=dif2.offset,
                            ap=[list(dif2.ap[0]), [1, 64 * M]])
        nc.vector.tensor_scalar(dif2_flat, dif2_flat, PDELTA, None, ALU.add)
        nc.vector.reciprocal(dif2, dif2)                       # R
        TT(SQ, S, dif, ALU.mult)                               # W = S*dif
        TT(SQ, SQ, dif2, ALU.mult)
        Wv = bass.AP(tensor=SQ.tensor, offset=SQ.offset,
                     ap=[list(SQ.ap[0]), [8 * M, 8], [1, M], [M, 8]])
        nc.vector.tensor_reduce(corr, Wv, mybir.AxisListType.X, ALU.add)
        TT(dv(h0, [[NM, 8], [1, M]]), dv(h0, [[NM, 8], [1, M]]), corr, ALU.add)

    def emit_sortassemble(hx):
        h0 = halves[hx][0]
        M = HM
        loc = {i: dv(i * NM + h0, [[1, M]]) for i in range(8)}
        tmin = main.tile([128, M], f32, tag=f"tmin{hx}", name="tmin")[:]
        ce = _CE8 if not (k0 == 4 and k1 == 4) else (_CE8[:16] + [(3, 4)])
        for (i, j) in ce:
            di, dj = loc[i], loc[j]
            TT(tmin, di, dj, ALU.min)
            TT(dj, di, dj, ALU.max)
            nc.vector.tensor_copy(di, tmin)

        def L(name):
            return main.tile([128, M], f32, tag=f"{name}{hx}", name=name)[:]

        mu_min = mn[:, h0:h0 + M]
        mu_max = mx[:, h0:h0 + M]
        nu_min = mn[:, NTILES + h0:NTILES + h0 + M]
        nu_max = mx[:, NTILES + h0:NTILES + h0 + M]
        acc = acc_full[:, h0:h0 + M]
        b0, b1 = L("b0"), L("b1")
        nc.vector.tensor_scalar(b0, loc[0], -8.0, 1.0, ALU.mult, ALU.add)
        nc.vector.reciprocal(b0, b0)
        nc.vector.tensor_scalar(b1, loc[7], -8.0, 1.0, ALU.mult, ALU.add)
        nc.vector.reciprocal(b1, b1)

        assert 1 <= k0 <= 8 and 1 <= k1 <= 8
        t1, t2_ = L("t1"), L("t2")
        sA, sB = L("sA"), L("sB")
        if k0 == 4:
            TT(sA, loc[0], loc[1], ALU.add)
            GT(sB, loc[2], loc[3], ALU.add)
            TT(t1, sA, sB, ALU.add)
        else:
            nc.gpsimd.tensor_copy(t1, loc[0])
            for i in range(1, k0):
                TT(t1, t1, loc[i], ALU.add)
        if k1 == 4:
            TT(sA, loc[7], loc[6], ALU.add)
            GT(sB, loc[5], loc[4], ALU.add)
            TT(t2_, sA, sB, ALU.add)
        else:
            nc.gpsimd.tensor_copy(t2_, loc[7])
            for i in range(6, 7 - k1, -1):
                TT(t2_, t2_, loc[i], ALU.add)
        u0, u1 = L("u0"), L("u1")
        STT(u0, t1, -k0 / 8.0, b0, ALU.add, ALU.mult)
        STT(u1, t2_, -k1 / 8.0, b1, ALU.add, ALU.mult)
        TT(u0, u0, u1, ALU.add)
        nc.vector.tensor_scalar(u0, u0, (k0 + k1) / 8.0, None, ALU.add)
        TT(acc, u0, u0, ALU.mult)
        t3s = [L(f"t3{i}") for i in range(4)]
        for n, (beta, ext) in enumerate(
                ((b0, mu_min), (b1, mu_max), (b0, nu_min), (b1, nu_max))):
            t3 = t3s[n]
            STT(t3, ext, -0.125, beta, ALU.add, ALU.mult)
            nc.vector.tensor_scalar(t3, t3, 0.125, None, ALU.add)
            if n % 2 == 0:
                TT(t3, t3, t3, ALU.mult)
            else:
                GT(t3, t3, t3, ALU.mult)
        TT(t3s[0], t3s[0], t3s[1], ALU.add)
        GT(t3s[2], t3s[2], t3s[3], ALU.add)
        TT(t3s[0], t3s[0], t3s[2], ALU.add)
        TT(acc, acc, t3s[0], ALU.add)

    emit_perturb(0)
    emit_perturb(1)
    emit_sortassemble(0)
    nc.sync.dma_start(out=out_d[:, 0:HM], in_=acc_full[:, 0:HM])
    emit_sortassemble(1)
    nc.sync.dma_start(out=out_d[:, HM:MRHO], in_=acc_full[:, HM:MRHO])


_prog_cache = {}


def kernel(rho_vec, rank0, rank1):
    rho_vec = np.asarray(rho_vec, dtype=np.float32)
    k0 = D - int(rank0)
    k1 = D - int(rank1)
    in_maps = _host_prep(rho_vec)

    from concourse.bass_utils import run_bass_kernel_spmd
    key = (k0, k1)
    if key not in _prog_cache:
        _prog_cache[key] = _build_program(k0, k1)
    nc = _prog_cache[key]
    res = run_bass_kernel_spmd(nc, in_maps, core_ids=list(range(NCORES)))
    return np.concatenate(
        [np.asarray(res.results[c]["out"]).T.reshape(-1) for c in range(NCORES)]
    ).astype(np.float32)
